# revision 28
# baseline (speedup 1.0000x reference)
import sys
if '/opt/trn_rl_repo' not in sys.path:
    sys.path.insert(0, '/opt/trn_rl_repo')
import numpy as np
import ml_dtypes

import concourse.bass as bass
import concourse.bacc as bacc
import concourse.tile as tile
from concourse import mybir
from concourse.bass_utils import run_bass_kernel_spmd

BF = mybir.dt.bfloat16
F32 = mybir.dt.float32
F8 = mybir.dt.float8e4
FN = mybir.ActivationFunctionType
OP = mybir.AluOpType
AX = mybir.AxisListType
DR = mybir.MatmulPerfMode.DoubleRow

B, N, F, T = 8, 2048, 2, 12
K, O = 3, 64
FT = F * T           # 24
NTILES = N // 128    # 16
OT = O * T           # 768
LN_EPS = 1e-5
SV = 8.0             # scale folded into fp8 Vs (S_pre = psum/(2*SV) + rowb)

bf16 = ml_dtypes.bfloat16
f8e4 = ml_dtypes.float8_e4m3

_CACHE = {}


def _build():
    nc = bacc.Bacc("TRN2", target_bir_lowering=False, debug=False, num_devices=8)

    def din(name, shape, dt=BF):
        return nc.declare_dram_parameter(name, list(shape), dt, isOutput=False)

    xnat = din("xnat", (128, NTILES * FT))        # x[b] [n,(f,t)] tiled: part=n%128
    xnatt = din("xnatt", (128, NTILES * FT))      # x[b] [n,(t,f)] tiled
    xftn = din("xftn", (FT, N))                   # [(f,t), n]
    xtfn = din("xtfn", (FT, N))                   # [(t,f), n]
    u1t = din("u1t", (128, NTILES))
    u2t = din("u2t", (128, NTILES * F))
    u3b = din("u3b", (128, 2), F32)
    w1 = din("w1", (T, 1))
    w2 = din("w2", (F, T))
    w3b = din("w3b", (128, 2), F32)
    bes = din("bes", (T, T), F32)
    vet = din("vet", (T, T), F32)
    ident = din("ident", (T, T), F32)
    bsb = din("bsb", (N, N), F8)                  # fp8 spatial-attn bias
    idDR = din("idDR", (128, 384), F8)            # [I|0|I] for DoubleRow bias-add
    idbf = din("idbf", (128, 128))                # identity bf16 (diag mask)
    vst = din("vst", (NTILES, 128, N), F8)        # per m-tile: SV*Vs^T fp8
    rowb = din("rowb", (128, NTILES), F32)        # 0.5*rowsum(Vs) per m
    chebb = din("chebb", (K - 1, N, N))           # cheb k=1,2 only (k=0 == I)
    thg = din("thg", (96, T * O))                 # zero-padded gcn lhsT per t
    twa = din("twa", (128, O))                    # [dt0|dt1] stacked tconv lhsT
    twb = din("twb", (88, T * O))                 # [dt2 ; res] per-t tconv lhsT
    etm = din("etm", (O, T * T))                  # block t: e_t col (LN mean)
    biaso = din("biaso", (O, 1), F32)
    gfull = din("gfull", (128, OT))               # ln_g in (t,o) order, bf16
    bfull = din("bfull", (128, OT))               # ln_b in (t,o) order, bf16
    onesb = din("onesb", (128, 1))

    y_d = nc.declare_dram_parameter("y", [N, OT], BF, isOutput=True)  # (t,o) order
    # per-512-column scratch chunks [(t,o)+stats, 512]: rows 768:780 carry the
    # per-(t,n) mean-sums so the transpose-DMA delivers them pre-transposed.
    YR = OT + 32  # 800, multiple of 32 for DmaTranspose
    ypd_c = [nc.dram_tensor("ypd%d" % c, [YR, 512], BF) for c in range(4)]

    with tile.TileContext(nc) as tc:
        from contextlib import ExitStack
        es = ExitStack()
        sm = es.enter_context(tc.tile_pool(name="sm", bufs=1))

        def load(dram, shape, dt=BF):
            t_ = sm.tile(list(shape), dt, tag=dram.name + "_s")
            nc.sync.dma_start(t_[:], dram[:])
            return t_

        xnat_s = load(xnat, (128, NTILES * FT))
        xnatt_s = load(xnatt, (128, NTILES * FT))

        xtfn_s = load(xtfn, (FT, N))
        u1t_s = load(u1t, (128, NTILES))
        u2t_s = load(u2t, (128, NTILES * F))
        u3b_s = load(u3b, (128, 2), F32)
        w1_s = load(w1, (T, 1))
        w2_s = load(w2, (F, T))
        w3b_s = load(w3b, (128, 2), F32)
        bes_s = load(bes, (T, T), F32)
        vet_s = load(vet, (T, T), F32)
        id_s = load(ident, (T, T), F32)
        idDR_s = load(idDR, (128, 384), F8)
        idb_s = load(idbf, (128, 128))
        rowb_s = load(rowb, (128, NTILES), F32)
        thg_s = load(thg, (96, T * O))
        twa_s = load(twa, (128, O))
        twb_s = load(twb, (88, T * O))
        etm_s = load(etm, (O, T * T))
        biaso_s = load(biaso, (O, 1), F32)
        ones_s = load(onesb, (128, 1))

        # ================= temporal attention =================
        rhss_s = sm.tile([T, N], BF, tag="rhss")
        l2t_s = sm.tile([T, N], BF, tag="l2t")
        with tc.tile_pool(name="pst", bufs=1, space="PSUM") as pst, \
             tc.tile_pool(name="pat", bufs=1, space="PSUM") as pat, \
             tc.tile_pool(name="attp", bufs=1) as attp:
            xftn_f = []
            for f in range(F):
                t_ = attp.tile([T, N], BF, tag="xftn%d" % f)
                nc.sync.dma_start(t_[:], xftn[f * T:(f + 1) * T, :])
                xftn_f.append(t_)
            # xU1 [1, 24] = sum_n U1[n] * x_nat[n, (f,t)]
            xu1_ps = pst.tile([1, FT], F32, tag="tiny_ps")
            for j in range(NTILES):
                nc.tensor.matmul(xu1_ps[:], u1t_s[:, j:j + 1],
                                 xnat_s[:, j * FT:(j + 1) * FT],
                                 start=(j == 0), stop=(j == NTILES - 1))
            xu1_ft = sm.tile([F, T], F32, tag="xu1ft")
            xu1_row = attp.tile([1, FT], F32, tag="xu1row")
            nc.vector.tensor_copy(xu1_row[:], xu1_ps[:])
            for f in range(F):
                nc.sync.dma_start(xu1_ft[f:f + 1, :],
                                  xu1_row[0:1, f * T:(f + 1) * T])

            # rhs_t [n, u] (packed [128, (j,u)]) = x[.,0,u]*U3[0] + x[.,1,u]*U3[1]
            rhs_t = attp.tile([128, NTILES * T], BF, tag="rhs_t")
            xf0 = xnat_s[:].rearrange("p (j f t) -> p j f t", j=NTILES, f=F, t=T)
            tmp_rt = attp.tile([128, NTILES * T], BF, tag="tmp_rt")
            rt_v = tmp_rt[:].rearrange("p (j t) -> p j t", j=NTILES, t=T)
            rhs_tv = rhs_t[:].rearrange("p (j t) -> p j t", j=NTILES, t=T)
            nc.vector.tensor_scalar_mul(rt_v, xf0[:, :, 1, :], u3b_s[:, 1:2])
            nc.vector.scalar_tensor_tensor(rhs_tv, xf0[:, :, 0, :], u3b_s[:, 0:1],
                                           rt_v, op0=OP.mult, op1=OP.add)

            # M [f, u] = sum_n U2^T-tiles @ rhs_t
            m_ps = pst.tile([F, T], F32, tag="tiny_ps")
            for j in range(NTILES):
                nc.tensor.matmul(m_ps[:], u2t_s[:, j * F:(j + 1) * F],
                                 rhs_t[:, j * T:(j + 1) * T],
                                 start=(j == 0), stop=(j == NTILES - 1))
            m_s = sm.tile([F, T], F32, tag="m_s")
            nc.vector.tensor_copy(m_s[:], m_ps[:])

            # prod_t [t, u] = xu1_ft^T @ M ; PT = sigmoid(prod_t + be)
            pt_ps = pst.tile([T, T], F32, tag="tiny_ps")
            nc.tensor.matmul(pt_ps[:], xu1_ft[:], m_s[:], start=True, stop=True)
            pt_sb = sm.tile([T, T], F32, tag="pt_sb")
            nc.vector.tensor_add(pt_sb[:], pt_ps[:], bes_s[:])
            nc.scalar.activation(pt_sb[:], pt_sb[:], FN.Sigmoid)

            # E0 [t, v] = Ve @ PT  (lhsT = Ve^T)
            e0_ps = pst.tile([T, T], F32, tag="tiny_ps")
            nc.tensor.matmul(e0_ps[:], vet_s[:], pt_sb[:], start=True, stop=True)
            e0_s = sm.tile([T, T], F32, tag="e0_s")
            nc.vector.tensor_copy(e0_s[:], e0_ps[:])
            e0t_ps = pst.tile([T, T], F32, tag="tiny_ps")
            nc.tensor.transpose(e0t_ps[:], e0_s[:], id_s[:])
            e0t_s = sm.tile([T, T], F32, tag="e0t_s")
            nc.vector.tensor_copy(e0t_s[:], e0t_ps[:])
            # softmax along free (t) for each v
            mx = sm.tile([T, 1], F32, tag="mx")
            nc.vector.tensor_reduce(mx[:], e0t_s[:], axis=AX.X, op=OP.max)
            nmx = sm.tile([T, 1], F32, tag="nmx")
            nc.vector.tensor_scalar_mul(nmx[:], mx[:], -1.0)
            esum = sm.tile([T, 1], F32, tag="esum")
            nc.scalar.activation(e0t_s[:], e0t_s[:], FN.Exp, bias=nmx[:],
                                 scale=1.0, accum_out=esum[:])
            recs = sm.tile([T, 1], F32, tag="recs")
            nc.vector.reciprocal(recs[:], esum[:])
            nc.vector.tensor_scalar_mul(e0t_s[:], e0t_s[:], recs[:])
            esm_ps = pst.tile([T, T], F32, tag="tiny_ps")
            nc.tensor.transpose(esm_ps[:], e0t_s[:], id_s[:])
            esm_bf = sm.tile([T, T], BF, tag="esm_bf")
            nc.vector.tensor_copy(esm_bf[:], esm_ps[:])

            # xTA_ftn [(f,t), n] : per f: Esm^T @ x_ftn[f]
            xta_f = []
            for f in range(F):
                xta_t = attp.tile([T, N], BF, tag="xta%d" % f)
                xta_ps = pat.tile([T, N], F32, tag="attnps")
                for c in range(4):
                    nc.tensor.matmul(xta_ps[:, c * 512:(c + 1) * 512], esm_bf[:],
                                     xftn_f[f][:, c * 512:(c + 1) * 512],
                                     start=True, stop=True)
                nc.vector.tensor_copy(xta_t[:], xta_ps[:])
                xta_f.append(xta_t)

            # ============== spatial attention (small parts) ==============
            tmp_rs = attp.tile([T, N], BF, tag="tmp_rs")
            nc.vector.tensor_scalar_mul(tmp_rs[:], xta_f[1][:],
                                        w3b_s[0:T, 1:2])
            nc.vector.scalar_tensor_tensor(rhss_s[:], xta_f[0][:],
                                           w3b_s[0:T, 0:1], tmp_rs[:],
                                           op0=OP.mult, op1=OP.add)
            lsfn_s = sm.tile([F, N], BF, tag="lsfn")
            for f in range(F):
                ls_ps = pat.tile([1, N], F32, tag="attnps")
                for c in range(4):
                    nc.tensor.matmul(ls_ps[:, c * 512:(c + 1) * 512], w1_s[:],
                                     xta_f[f][:, c * 512:(c + 1) * 512],
                                     start=True, stop=True)
                if f == 0:
                    nc.vector.tensor_copy(lsfn_s[0:1, :], ls_ps[:])
                else:
                    ls1 = attp.tile([1, N], BF, tag="ls1")
                    nc.vector.tensor_copy(ls1[:], ls_ps[:])
                    nc.sync.dma_start(lsfn_s[1:2, :], ls1[:])
            l2_ps = pat.tile([T, N], F32, tag="attnps")
            for c in range(4):
                nc.tensor.matmul(l2_ps[:, c * 512:(c + 1) * 512], w2_s[:],
                                 lsfn_s[:, c * 512:(c + 1) * 512],
                                 start=True, stop=True)
            nc.vector.tensor_copy(l2t_s[:], l2_ps[:])

        # ========== Pc = tanh(0.5*(prod_s + bs)) in fp8 ==========
        from contextlib import ExitStack as _ES
        es2 = _ES()
        midp = es2.enter_context(tc.tile_pool(name="midp", bufs=1))
        rec_s = midp.tile([1, N], BF, tag="rec_s")
        rec_b = midp.tile([FT, N], BF, tag="rec_b")
        z_sb = midp.tile([96, N], BF, tag="z_sb")
        NPRE = 8  # cheb tiles prefetched into SBUF during the P phase
        with tc.tile_pool(name="Epool", bufs=1) as epool:
            E_s = epool.tile([128, NTILES * N], BF, tag="E")
            chpre = [epool.tile([128, N], BF, tag="chpre%d" % i,
                                name="chpre%d" % i)
                     for i in range(NPRE)]
            with tc.tile_pool(name="Ppool", bufs=1) as ppool:
                Pc_s = ppool.tile([128, NTILES * N], F8, tag="Pc")
                with tc.tile_pool(name="bsstr", bufs=3) as bsstr, \
                     tc.tile_pool(name="pps", bufs=2, space="PSUM") as pps:
                    for nt in range(NTILES):
                        bst = bsstr.tile([128, N], F8, tag="bst")
                        nc.sync.dma_start(bst[:], bsb[nt * 128:(nt + 1) * 128, :])
                        if nt < NPRE:  # prefetch cheb k=1 while DMA is idle
                            nc.sync.dma_start(
                                chpre[nt][:], chebb[0, nt * 128:(nt + 1) * 128, :])
                        pr_ps = pps.tile([128, N], F32, tag="pr_ps")
                        for c in range(4):
                            nc.tensor.matmul(pr_ps[:, c * 512:(c + 1) * 512],
                                             l2t_s[:, nt * 128:(nt + 1) * 128],
                                             rhss_s[:, c * 512:(c + 1) * 512],
                                             start=True, stop=False)
                            # DoubleRow identity-add of bs: lhsT [I|0] or [0|I]
                            base = (c // 2) * 1024
                            idv = idDR_s[:, (c % 2) * 128:(c % 2) * 128 + 256]
                            nc.tensor.matmul(
                                pr_ps[:, c * 512:(c + 1) * 512],
                                idv.rearrange("p (j m) -> p j m", j=2, m=128),
                                bst[:, base:base + 1024].rearrange(
                                    "p (j n) -> p j n", j=2, n=512),
                                start=False, stop=True, perf_mode=DR)
                        nc.scalar.activation(Pc_s[:, nt * N:(nt + 1) * N],
                                             pr_ps[:], FN.Tanh, scale=0.5)

                # == S_pre = SV*Vs @ Pc (fp8 DoubleRow); E = exp(psum/(2SV)+rowb)
                # colsum via DVE accumulate + one GPSIMD partition all-reduce.
                pc_v = Pc_s[:].rearrange("p (j n) -> p j n", j=NTILES, n=N)
                with tc.tile_pool(name="vstr", bufs=4) as vstr, \
                     tc.tile_pool(name="accp", bufs=1) as accp, \
                     tc.tile_pool(name="sps", bufs=4, space="PSUM") as sps:
                    acc_s = accp.tile([128, N], F32, tag="acc")
                    cs_s = accp.tile([128, N], F32, tag="cs")
                    for mt in range(NTILES):
                        vt = vstr.tile([128, N], F8, tag="vt")
                        nc.sync.dma_start(vt[:], vst[mt, :, :])
                        vt_v = vt[:].rearrange("p (j m) -> p j m", j=NTILES, m=128)
                        for h in range(2):
                            s_ps = sps.tile([128, N // 2], F32, tag="s_ps")
                            for c in range(2):
                                col = h * 1024 + c * 512
                                for jp in range(NTILES // 2):
                                    nc.tensor.matmul(
                                        s_ps[:, c * 512:(c + 1) * 512],
                                        vt_v[:, 2 * jp:2 * jp + 2, :],
                                        pc_v[:, 2 * jp:2 * jp + 2, col:col + 512],
                                        start=(jp == 0),
                                        stop=(jp == NTILES // 2 - 1),
                                        perf_mode=DR)
                            ev = E_s[:, mt * N + h * 1024:mt * N + (h + 1) * 1024]
                            nc.scalar.activation(ev, s_ps[:], FN.Exp,
                                                 bias=rowb_s[:, mt:mt + 1],
                                                 scale=1.0 / (2.0 * SV))
                        if mt == 0:
                            nc.vector.tensor_copy(
                                acc_s[:], E_s[:, 0:N])
                        else:
                            nc.vector.tensor_add(
                                acc_s[:], acc_s[:], E_s[:, mt * N:(mt + 1) * N])
                    import concourse.bass_isa as bass_isa
                    nc.gpsimd.partition_all_reduce(
                        cs_s[:], acc_s[:], channels=128,
                        reduce_op=bass_isa.ReduceOp.add)
                    with nc.allow_low_precision(reason="colsum ~2048*E; bf16 "
                                                "rec only scales softmax"):
                        nc.vector.reciprocal(rec_s[:], cs_s[0:1, :])
                nc.gpsimd.partition_broadcast(rec_b[:], rec_s[:])

            # ========== cheb: Z_k = x_natt^T @ (cheb_k * E) ==========
            # k=0: cheb_0 == I, so only diagonal blocks of E contribute.
            nc.vector.memset(z_sb[:], 0.0)
            with tc.tile_pool(name="chstr", bufs=6) as chstr, \
                 tc.tile_pool(name="wtmp", bufs=6) as wtmp, \
                 tc.tile_pool(name="zps", bufs=2, space="PSUM") as zps:
                z0_ps = zps.tile([FT, N], F32, tag="z_ps")
                for mt in range(NTILES):
                    wt0 = wtmp.tile([128, 128], BF, tag="wt0")
                    d0 = mt * N + mt * 128
                    nc.vector.tensor_mul(wt0[:], E_s[:, d0:d0 + 128], idb_s[:])
                    nc.tensor.matmul(z0_ps[:, mt * 128:(mt + 1) * 128],
                                     xnatt_s[:, mt * FT:(mt + 1) * FT],
                                     wt0[:], start=True, stop=True)
                nc.vector.tensor_mul(z_sb[0:FT, :], z0_ps[:], rec_b[:])
                for k in range(1, K):
                    z_ps = zps.tile([FT, N], F32, tag="z_ps")
                    for mt in range(NTILES):
                        if k == 1 and mt < NPRE:
                            cht_ap = chpre[mt][:]
                        else:
                            cht = chstr.tile([128, N], BF, tag="cht")
                            nc.sync.dma_start(
                                cht[:], chebb[k - 1, mt * 128:(mt + 1) * 128, :])
                            cht_ap = cht[:]
                        wt = wtmp.tile([128, N], BF, tag="wt")
                        eng = nc.gpsimd if (mt % 4) == 3 else nc.vector
                        eng.tensor_mul(wt[:], cht_ap,
                                       E_s[:, mt * N:(mt + 1) * N])
                        for c in range(4):
                            nc.tensor.matmul(
                                z_ps[:, c * 512:(c + 1) * 512],
                                xnatt_s[:, mt * FT:(mt + 1) * FT],
                                wt[:, c * 512:(c + 1) * 512],
                                start=(mt == 0), stop=(mt == NTILES - 1))
                    nc.vector.tensor_mul(z_sb[32 * k:32 * k + FT, :], z_ps[:],
                                         rec_b[:])

        # ================= gcn =================
        with tc.tile_pool(name="gcnp", bufs=1) as gcnp, \
             tc.tile_pool(name="ypp", bufs=2) as ypp, \
             tc.tile_pool(name="ynp", bufs=4) as ynp, \
             tc.tile_pool(name="lns", bufs=4) as lns:
            # gcnB rows 0-63: gcn_pad blocks 0..13 ([0,g0..g11,0]); rows
            # 64-127: gcn_pad blocks shifted by one ([g0..g11,0,-]).
            gcnB = gcnp.tile([128, (T + 2) * N], BF, tag="gcnB")
            # gcnC rows 0-63: gcn_pad blocks shifted by two; rows 64-87: xtfn
            # replicated at every t block (residual-conv input).
            gcnC = gcnp.tile([88, T * N], BF, tag="gcnC")
            nc.vector.memset(gcnB[0:64, 0:N], 0.0)
            nc.vector.memset(gcnB[0:64, (T + 1) * N:(T + 2) * N], 0.0)
            for t in range(T):
                nc.sync.dma_start(gcnC[64:88, t * N:(t + 1) * N], xtfn_s[:])
            with tc.tile_pool(name="gps", bufs=2, space="PSUM") as gps:
                for t in range(T):
                    g_ps = gps.tile([O, N], F32, tag="g_ps")
                    for c in range(4):
                        nc.tensor.matmul(
                            g_ps[:, c * 512:(c + 1) * 512],
                            thg_s[:, t * O:(t + 1) * O],
                            z_sb[:, c * 512:(c + 1) * 512],
                            start=True, stop=True)
                    nc.scalar.activation(gcnB[0:64, (t + 1) * N:(t + 2) * N],
                                         g_ps[:], FN.Relu)
                    # shifted copies for the packed-contraction tconv
                    nc.sync.dma_start(gcnB[64:128, t * N:(t + 1) * N],
                                      gcnB[0:64, (t + 1) * N:(t + 2) * N])
                    if t >= 2:
                        nc.sync.dma_start(
                            gcnC[0:64, (t - 2) * N:(t - 1) * N],
                            gcnB[0:64, t * N:(t + 1) * N])
            for t in (T - 2, T - 1):
                nc.sync.dma_start(gcnC[0:64, t * N:(t + 1) * N],
                                  gcnB[0:64, (t + 2) * N:(t + 3) * N])

            # ====== tconv + res, with layernorm interleaved per column-chunk
            gfull_s = lns.tile([128, OT], BF, tag="gfull_s")
            nc.sync.dma_start(gfull_s[:], gfull[:])
            bfull_s = lns.tile([128, OT], BF, tag="bfull_s")
            nc.sync.dma_start(bfull_s[:], bfull[:])
            epsb = lns.tile([128, 1], F32, tag="epsb")
            nc.vector.memset(epsb[:], float(O) * LN_EPS)
            with tc.tile_pool(name="tps", bufs=6, space="PSUM") as tps, \
                 tc.tile_pool(name="stp", bufs=2, space="PSUM") as stp:
                for c in range(4):
                    ypre_s = ypp.tile([O, T * 512], BF, tag="ypre")
                    st_ps = stp.tile([T, 512], F32, tag="st_ps")
                    for t in range(T):
                        tc_ps = tps.tile([O, 512], F32, tag="tc_ps")
                        nc.tensor.matmul(
                            tc_ps[:], twa_s[:],
                            gcnB[:, t * N + c * 512:t * N + (c + 1) * 512],
                            start=True, stop=False)
                        nc.tensor.matmul(
                            tc_ps[:], twb_s[:, t * O:(t + 1) * O],
                            gcnC[:, t * N + c * 512:t * N + (c + 1) * 512],
                            start=False, stop=True)
                        nc.scalar.activation(
                            ypre_s[:, t * 512:(t + 1) * 512],
                            tc_ps[:], FN.Relu, bias=biaso_s[:], scale=1.0)
                        # accumulate per-(t,n) mean-sums on PE
                        nc.tensor.matmul(
                            st_ps[:], etm_s[:, t * T:(t + 1) * T],
                            ypre_s[:, t * 512:(t + 1) * 512],
                            start=(t == 0), stop=(t == T - 1))
                    st_sb = lns.tile([T, 512], BF, tag="st_sb")
                    nc.vector.tensor_copy(st_sb[:], st_ps[:])
                    ypdv = ypd_c[c][0:OT, :].rearrange("(t o) n -> o t n",
                                                       o=O, t=T)
                    nc.sync.dma_start(
                        ypdv[:, :, :],
                        ypre_s[:].rearrange("o (t n) -> o t n", t=T))
                    nc.sync.dma_start(ypd_c[c][OT:OT + T, :], st_sb[:])
                    # ---- layernorm for the 4 n-tiles of this chunk ----
                    for q in range(4):
                        nt = c * 4 + q
                        yt = ynp.tile([128, YR], BF, tag="yt")
                        nc.sync.dma_start_transpose(
                            yt[:], ypd_c[c][:, q * 128:(q + 1) * 128])
                        ytv = yt[:, 0:OT].rearrange("p (t o) -> p t o",
                                                    o=O, t=T)
                        mus = yt[:, OT:OT + T]           # bf16 mean-sums
                        mud = lns.tile([128, T], F32, tag="mud")
                        nc.gpsimd.tensor_scalar_mul(mud[:], mus, 1.0 / O)
                        sqf = ynp.tile([128, OT], BF, tag="sqf")
                        seng = nc.vector if (q % 2) == 0 else nc.gpsimd
                        seng.tensor_mul(sqf[:], yt[:, 0:OT], yt[:, 0:OT])
                        sqs = lns.tile([128, T], F32, tag="sqs")
                        nc.vector.tensor_reduce(
                            sqs[:], sqf[:].rearrange("p (t o) -> p t o",
                                                     o=O, t=T),
                            axis=AX.X, op=OP.add)
                        w_t = lns.tile([128, T], F32, tag="w_t")
                        nc.gpsimd.tensor_mul(w_t[:], mus, mud[:])
                        nc.gpsimd.tensor_sub(w_t[:], sqs[:], w_t[:])  # 64*var
                        s_t = lns.tile([128, T], F32, tag="s_t")
                        nc.scalar.activation(s_t[:], w_t[:], FN.Sqrt,
                                             bias=epsb[:])
                        r_t = lns.tile([128, T], F32, tag="r_t")
                        nc.vector.reciprocal(r_t[:], s_t[:])  # isig/8
                        nmr = lns.tile([128, T], F32, tag="nmr")
                        nc.gpsimd.tensor_scalar_mul(nmr[:], mud[:], -1.0)
                        nc.gpsimd.tensor_mul(nmr[:], nmr[:], r_t[:])
                        yh = ynp.tile([128, OT], BF, tag="yh")
                        yhv = yh[:].rearrange("p (t o) -> p t o", o=O, t=T)
                        for t in range(T):
                            nc.scalar.activation(
                                yhv[:, t, :], ytv[:, t, :], FN.Identity,
                                bias=nmr[:, t:t + 1], scale=r_t[:, t:t + 1])
                        yg = ynp.tile([128, OT], BF, tag="yg")
                        nc.vector.tensor_mul(yg[:], yh[:], gfull_s[:])
                        aeng = nc.vector if (q % 2) == 0 else nc.gpsimd
                        aeng.tensor_add(yg[:], yg[:], bfull_s[:])
                        nc.sync.dma_start(y_d[nt * 128:(nt + 1) * 128, :],
                                          yg[:])
        es2.close()

        es.close()
    nc.compile()
    return nc


def _pack(x_b, consts):
    m = dict(consts)
    xb = np.asarray(x_b, np.float32)
    x_nat = xb.reshape(N, FT)
    x_natt = np.ascontiguousarray(xb.transpose(0, 2, 1)).reshape(N, FT)
    pk = lambda a: np.ascontiguousarray(
        a.reshape(NTILES, 128, FT).transpose(1, 0, 2).reshape(128, NTILES * FT)
    ).astype(bf16)
    m["xnat"] = pk(x_nat)
    m["xnatt"] = pk(x_natt)
    m["xftn"] = np.ascontiguousarray(xb.transpose(1, 2, 0).reshape(FT, N)).astype(bf16)
    m["xtfn"] = np.ascontiguousarray(xb.transpose(2, 1, 0).reshape(FT, N)).astype(bf16)
    return m


def kernel(**inputs):
    if "nc" not in _CACHE:
        _CACHE["nc"] = _build()
    nc = _CACHE["nc"]

    f32 = lambda a: np.asarray(a, np.float32)
    x = f32(inputs["x"])
    U1, U2, U3 = f32(inputs["U1"]), f32(inputs["U2"]), f32(inputs["U3"])
    Vs, bs = f32(inputs["Vs"]), f32(inputs["bs"])
    cheb = f32(inputs["cheb"])
    Theta = f32(inputs["Theta"])
    W1, W2, W3 = f32(inputs["W1"]), f32(inputs["W2"]), f32(inputs["W3"])
    be, Ve = f32(inputs["be"]), f32(inputs["Ve"])
    tw, tb = f32(inputs["tconv_w"]), f32(inputs["tconv_b"])
    rw, rb = f32(inputs["rconv_w"]), f32(inputs["rconv_b"])
    g, bb = f32(inputs["ln_g"]), f32(inputs["ln_b"])

    consts = {}
    consts["u1t"] = np.ascontiguousarray(U1.reshape(NTILES, 128).T).astype(bf16)
    consts["u2t"] = np.ascontiguousarray(
        U2.T.reshape(NTILES, 128, F).transpose(1, 0, 2).reshape(128, NTILES * F)
    ).astype(bf16)
    consts["u3b"] = np.broadcast_to(U3[None, :], (128, 2)).copy()
    consts["w1"] = W1[:, None].astype(bf16)
    consts["w2"] = W2.astype(bf16)
    consts["w3b"] = np.broadcast_to(W3[None, :], (128, 2)).copy()
    consts["bes"] = np.ascontiguousarray(be[0])
    consts["vet"] = np.ascontiguousarray(Ve.T)
    consts["ident"] = np.eye(T, dtype=np.float32)
    idDR = np.zeros((128, 384), np.float32)
    idDR[:, 0:128] = np.eye(128)
    idDR[:, 256:384] = np.eye(128)
    consts["idDR"] = idDR.astype(f8e4)
    consts["idbf"] = np.eye(128, dtype=np.float32).astype(bf16)
    consts["bsb"] = bs[0].astype(f8e4)
    VsT = np.ascontiguousarray(Vs.T)  # [n, m]
    consts["vst"] = np.ascontiguousarray(
        (SV * VsT).reshape(NTILES, 128, NTILES, 128).transpose(2, 1, 0, 3)
        .reshape(NTILES, 128, N)
    ).astype(f8e4)
    consts["rowb"] = np.ascontiguousarray(
        (0.5 * Vs.sum(axis=1)).reshape(NTILES, 128).T).astype(np.float32)
    consts["chebb"] = cheb[1:].astype(bf16)
    thg = np.zeros((96, T * O), np.float32)
    for t in range(T):
        for k in range(K):
            for f in range(F):
                thg[32 * k + t * F + f, t * O:(t + 1) * O] = Theta[k, f]
    consts["thg"] = thg.astype(bf16)
    # twa: [tw_dt0 ; tw_dt1] stacked along contraction; twb: per-t
    # [tw_dt2 (64 rows) ; residual-conv rows (24)]
    twl = tw[:, :, 0, :].transpose(1, 2, 0).reshape(O, 3 * O)  # [i, (dt, o)]
    consts["twa"] = np.concatenate(
        [twl[:, 0:O], twl[:, O:2 * O]], axis=0).astype(bf16)
    twb = np.zeros((88, T * O), np.float32)
    for t in range(T):
        twb[0:O, t * O:(t + 1) * O] = twl[:, 2 * O:3 * O]
        for f in range(F):
            twb[O + t * F + f, t * O:(t + 1) * O] = rw[:, f, 0, 0]
    consts["twb"] = twb.astype(bf16)
    etm_np = np.zeros((O, T * T), np.float32)
    for t in range(T):
        etm_np[:, t * T + t] = 1.0
    consts["etm"] = etm_np.astype(bf16)
    consts["biaso"] = (tb + rb)[:, None]
    consts["gfull"] = np.broadcast_to(
        (8.0 * np.tile(g, T))[None, :], (128, OT)).astype(bf16).copy()
    consts["bfull"] = np.broadcast_to(
        np.tile(bb, T)[None, :], (128, OT)).astype(bf16).copy()
    consts["onesb"] = np.ones((128, 1), bf16)

    in_maps = [_pack(x[b], consts) for b in range(B)]
    _CACHE["im"] = in_maps
    res = run_bass_kernel_spmd(nc, in_maps, list(range(B)))
    out = np.stack([
        np.asarray(res.results[b]["y"], np.float32)
        .reshape(N, T, O).transpose(0, 2, 1)
        for b in range(B)
    ])
    return out.astype(np.float32)


# revision 32
# speedup vs baseline: 1.0040x; 1.0040x over previous
import sys
if '/opt/trn_rl_repo' not in sys.path:
    sys.path.insert(0, '/opt/trn_rl_repo')
import numpy as np
import ml_dtypes

import concourse.bass as bass
import concourse.bacc as bacc
import concourse.tile as tile
from concourse import mybir
from concourse.bass_utils import run_bass_kernel_spmd

BF = mybir.dt.bfloat16
F32 = mybir.dt.float32
F8 = mybir.dt.float8e4
FN = mybir.ActivationFunctionType
OP = mybir.AluOpType
AX = mybir.AxisListType
DR = mybir.MatmulPerfMode.DoubleRow

B, N, F, T = 8, 2048, 2, 12
K, O = 3, 64
FT = F * T           # 24
NTILES = N // 128    # 16
OT = O * T           # 768
LN_EPS = 1e-5
SV = 8.0             # scale folded into fp8 Vs (S_pre = psum/(2*SV) + rowb)

bf16 = ml_dtypes.bfloat16
f8e4 = ml_dtypes.float8_e4m3

_CACHE = {}


def _build():
    nc = bacc.Bacc("TRN2", target_bir_lowering=False, debug=False, num_devices=8)

    def din(name, shape, dt=BF):
        return nc.declare_dram_parameter(name, list(shape), dt, isOutput=False)

    xnat = din("xnat", (128, NTILES * FT))        # x[b] [n,(f,t)] tiled: part=n%128
    xnatt = din("xnatt", (128, NTILES * FT))      # x[b] [n,(t,f)] tiled
    xftn = din("xftn", (FT, N))                   # [(f,t), n]
    xtfn = din("xtfn", (FT, N))                   # [(t,f), n]
    u1t = din("u1t", (128, NTILES))
    u2t = din("u2t", (128, NTILES * F))
    u3b = din("u3b", (128, 2), F32)
    w1 = din("w1", (T, 1))
    w2 = din("w2", (F, T))
    w3b = din("w3b", (128, 2), F32)
    bes = din("bes", (T, T), F32)
    vet = din("vet", (T, T), F32)
    ident = din("ident", (T, T), F32)
    bsb = din("bsb", (N, N), F8)                  # fp8 spatial-attn bias
    idDR = din("idDR", (128, 384), F8)            # [I|0|I] for DoubleRow bias-add
    idbf = din("idbf", (128, 128))                # identity bf16 (diag mask)
    vst = din("vst", (NTILES, 128, N), F8)        # per m-tile: SV*Vs^T fp8
    rowb = din("rowb", (128, NTILES), F32)        # 0.5*rowsum(Vs) per m
    chebb = din("chebb", (K - 1, N, N))           # cheb k=1,2 only (k=0 == I)
    thg = din("thg", (96, T * O))                 # zero-padded gcn lhsT per t
    twa = din("twa", (128, O))                    # [dt0|dt1] stacked tconv lhsT
    twb = din("twb", (88, T * O))                 # [dt2 ; res] per-t tconv lhsT
    etm = din("etm", (O, T * T))                  # block t: e_t col (LN mean)
    biaso = din("biaso", (O, 1), F32)
    gfull = din("gfull", (128, OT))               # ln_g in (t,o) order, bf16
    bfull = din("bfull", (128, OT))               # ln_b in (t,o) order, bf16
    onesb = din("onesb", (128, 1))

    y_d = nc.declare_dram_parameter("y", [N, OT], BF, isOutput=True)  # (t,o) order
    # per-512-column scratch chunks [(t,o)+stats, 512]: rows 768:780 carry the
    # per-(t,n) mean-sums so the transpose-DMA delivers them pre-transposed.
    YR = OT + 32  # 800, multiple of 32 for DmaTranspose
    ypd_c = [nc.dram_tensor("ypd%d" % c, [YR, 512], BF) for c in range(4)]

    with tile.TileContext(nc) as tc:
        from contextlib import ExitStack
        es = ExitStack()
        sm = es.enter_context(tc.tile_pool(name="sm", bufs=1))

        def load(dram, shape, dt=BF):
            t_ = sm.tile(list(shape), dt, tag=dram.name + "_s")
            nc.sync.dma_start(t_[:], dram[:])
            return t_

        xnat_s = load(xnat, (128, NTILES * FT))
        xnatt_s = load(xnatt, (128, NTILES * FT))

        xtfn_s = load(xtfn, (FT, N))
        u1t_s = load(u1t, (128, NTILES))
        u2t_s = load(u2t, (128, NTILES * F))
        u3b_s = load(u3b, (128, 2), F32)
        w1_s = load(w1, (T, 1))
        w2_s = load(w2, (F, T))
        w3b_s = load(w3b, (128, 2), F32)
        bes_s = load(bes, (T, T), F32)
        vet_s = load(vet, (T, T), F32)
        id_s = load(ident, (T, T), F32)
        idDR_s = load(idDR, (128, 384), F8)
        idb_s = load(idbf, (128, 128))
        rowb_s = load(rowb, (128, NTILES), F32)
        thg_s = load(thg, (96, T * O))
        twa_s = load(twa, (128, O))
        twb_s = load(twb, (88, T * O))
        etm_s = load(etm, (O, T * T))
        biaso_s = load(biaso, (O, 1), F32)
        ones_s = load(onesb, (128, 1))

        # ================= temporal attention =================
        rhss_s = sm.tile([T, N], BF, tag="rhss")
        l2t_s = sm.tile([T, N], BF, tag="l2t")
        with tc.tile_pool(name="pst", bufs=1, space="PSUM") as pst, \
             tc.tile_pool(name="pat", bufs=1, space="PSUM") as pat, \
             tc.tile_pool(name="attp", bufs=1) as attp:
            xftn_f = []
            for f in range(F):
                t_ = attp.tile([T, N], BF, tag="xftn%d" % f)
                nc.sync.dma_start(t_[:], xftn[f * T:(f + 1) * T, :])
                xftn_f.append(t_)
            # xU1 [1, 24] = sum_n U1[n] * x_nat[n, (f,t)]
            xu1_ps = pst.tile([1, FT], F32, tag="tiny_ps")
            for j in range(NTILES):
                nc.tensor.matmul(xu1_ps[:], u1t_s[:, j:j + 1],
                                 xnat_s[:, j * FT:(j + 1) * FT],
                                 start=(j == 0), stop=(j == NTILES - 1))
            xu1_ft = sm.tile([F, T], F32, tag="xu1ft")
            xu1_row = attp.tile([1, FT], F32, tag="xu1row")
            nc.vector.tensor_copy(xu1_row[:], xu1_ps[:])
            for f in range(F):
                nc.sync.dma_start(xu1_ft[f:f + 1, :],
                                  xu1_row[0:1, f * T:(f + 1) * T])

            # rhs_t [n, u] (packed [128, (j,u)]) = x[.,0,u]*U3[0] + x[.,1,u]*U3[1]
            rhs_t = attp.tile([128, NTILES * T], BF, tag="rhs_t")
            xf0 = xnat_s[:].rearrange("p (j f t) -> p j f t", j=NTILES, f=F, t=T)
            tmp_rt = attp.tile([128, NTILES * T], BF, tag="tmp_rt")
            rt_v = tmp_rt[:].rearrange("p (j t) -> p j t", j=NTILES, t=T)
            rhs_tv = rhs_t[:].rearrange("p (j t) -> p j t", j=NTILES, t=T)
            nc.vector.tensor_scalar_mul(rt_v, xf0[:, :, 1, :], u3b_s[:, 1:2])
            nc.vector.scalar_tensor_tensor(rhs_tv, xf0[:, :, 0, :], u3b_s[:, 0:1],
                                           rt_v, op0=OP.mult, op1=OP.add)

            # M [f, u] = sum_n U2^T-tiles @ rhs_t
            m_ps = pst.tile([F, T], F32, tag="tiny_ps")
            for j in range(NTILES):
                nc.tensor.matmul(m_ps[:], u2t_s[:, j * F:(j + 1) * F],
                                 rhs_t[:, j * T:(j + 1) * T],
                                 start=(j == 0), stop=(j == NTILES - 1))
            m_s = sm.tile([F, T], F32, tag="m_s")
            nc.vector.tensor_copy(m_s[:], m_ps[:])

            # prod_t [t, u] = xu1_ft^T @ M ; PT = sigmoid(prod_t + be)
            pt_ps = pst.tile([T, T], F32, tag="tiny_ps")
            nc.tensor.matmul(pt_ps[:], xu1_ft[:], m_s[:], start=True, stop=True)
            pt_sb = sm.tile([T, T], F32, tag="pt_sb")
            nc.vector.tensor_add(pt_sb[:], pt_ps[:], bes_s[:])
            nc.scalar.activation(pt_sb[:], pt_sb[:], FN.Sigmoid)

            # E0 [t, v] = Ve @ PT  (lhsT = Ve^T)
            e0_ps = pst.tile([T, T], F32, tag="tiny_ps")
            nc.tensor.matmul(e0_ps[:], vet_s[:], pt_sb[:], start=True, stop=True)
            e0_s = sm.tile([T, T], F32, tag="e0_s")
            nc.vector.tensor_copy(e0_s[:], e0_ps[:])
            e0t_ps = pst.tile([T, T], F32, tag="tiny_ps")
            nc.tensor.transpose(e0t_ps[:], e0_s[:], id_s[:])
            e0t_s = sm.tile([T, T], F32, tag="e0t_s")
            nc.vector.tensor_copy(e0t_s[:], e0t_ps[:])
            # softmax along free (t) for each v
            mx = sm.tile([T, 1], F32, tag="mx")
            nc.vector.tensor_reduce(mx[:], e0t_s[:], axis=AX.X, op=OP.max)
            nmx = sm.tile([T, 1], F32, tag="nmx")
            nc.vector.tensor_scalar_mul(nmx[:], mx[:], -1.0)
            esum = sm.tile([T, 1], F32, tag="esum")
            nc.scalar.activation(e0t_s[:], e0t_s[:], FN.Exp, bias=nmx[:],
                                 scale=1.0, accum_out=esum[:])
            recs = sm.tile([T, 1], F32, tag="recs")
            nc.vector.reciprocal(recs[:], esum[:])
            nc.vector.tensor_scalar_mul(e0t_s[:], e0t_s[:], recs[:])
            esm_ps = pst.tile([T, T], F32, tag="tiny_ps")
            nc.tensor.transpose(esm_ps[:], e0t_s[:], id_s[:])
            esm_bf = sm.tile([T, T], BF, tag="esm_bf")
            nc.vector.tensor_copy(esm_bf[:], esm_ps[:])

            # xTA_ftn [(f,t), n] : per f: Esm^T @ x_ftn[f]
            xta_f = []
            for f in range(F):
                xta_t = attp.tile([T, N], BF, tag="xta%d" % f)
                xta_ps = pat.tile([T, N], F32, tag="attnps")
                for c in range(4):
                    nc.tensor.matmul(xta_ps[:, c * 512:(c + 1) * 512], esm_bf[:],
                                     xftn_f[f][:, c * 512:(c + 1) * 512],
                                     start=True, stop=True)
                nc.vector.tensor_copy(xta_t[:], xta_ps[:])
                xta_f.append(xta_t)

            # ============== spatial attention (small parts) ==============
            tmp_rs = attp.tile([T, N], BF, tag="tmp_rs")
            nc.vector.tensor_scalar_mul(tmp_rs[:], xta_f[1][:],
                                        w3b_s[0:T, 1:2])
            nc.vector.scalar_tensor_tensor(rhss_s[:], xta_f[0][:],
                                           w3b_s[0:T, 0:1], tmp_rs[:],
                                           op0=OP.mult, op1=OP.add)
            lsfn_s = sm.tile([F, N], BF, tag="lsfn")
            for f in range(F):
                ls_ps = pat.tile([1, N], F32, tag="attnps")
                for c in range(4):
                    nc.tensor.matmul(ls_ps[:, c * 512:(c + 1) * 512], w1_s[:],
                                     xta_f[f][:, c * 512:(c + 1) * 512],
                                     start=True, stop=True)
                if f == 0:
                    nc.vector.tensor_copy(lsfn_s[0:1, :], ls_ps[:])
                else:
                    ls1 = attp.tile([1, N], BF, tag="ls1")
                    nc.vector.tensor_copy(ls1[:], ls_ps[:])
                    nc.sync.dma_start(lsfn_s[1:2, :], ls1[:])
            l2_ps = pat.tile([T, N], F32, tag="attnps")
            for c in range(4):
                nc.tensor.matmul(l2_ps[:, c * 512:(c + 1) * 512], w2_s[:],
                                 lsfn_s[:, c * 512:(c + 1) * 512],
                                 start=True, stop=True)
            nc.vector.tensor_copy(l2t_s[:], l2_ps[:])

        # ========== Pc = tanh(0.5*(prod_s + bs)) in fp8 ==========
        from contextlib import ExitStack as _ES
        es2 = _ES()
        midp = es2.enter_context(tc.tile_pool(name="midp", bufs=1))
        rec_s = midp.tile([1, N], BF, tag="rec_s")
        rec_b = midp.tile([FT, N], BF, tag="rec_b")
        z_sb = midp.tile([96, N], BF, tag="z_sb")
        NPRE = 8  # cheb tiles prefetched into SBUF during the P phase
        with tc.tile_pool(name="Epool", bufs=1) as epool:
            E_s = epool.tile([128, NTILES * N], BF, tag="E")
            chpre = [epool.tile([128, N], BF, tag="chpre%d" % i,
                                name="chpre%d" % i)
                     for i in range(NPRE)]
            with tc.tile_pool(name="Ppool", bufs=1) as ppool:
                Pc_s = ppool.tile([128, NTILES * N], F8, tag="Pc")
                with tc.tile_pool(name="bsstr", bufs=3) as bsstr, \
                     tc.tile_pool(name="pps", bufs=2, space="PSUM") as pps:
                    for nt in range(NTILES):
                        bst = bsstr.tile([128, N], F8, tag="bst")
                        nc.sync.dma_start(bst[:], bsb[nt * 128:(nt + 1) * 128, :])
                        if nt < NPRE:  # prefetch cheb k=1 while DMA is idle
                            nc.sync.dma_start(
                                chpre[nt][:], chebb[0, nt * 128:(nt + 1) * 128, :])
                        pr_ps = pps.tile([128, N], F32, tag="pr_ps")
                        for c in range(4):
                            nc.tensor.matmul(pr_ps[:, c * 512:(c + 1) * 512],
                                             l2t_s[:, nt * 128:(nt + 1) * 128],
                                             rhss_s[:, c * 512:(c + 1) * 512],
                                             start=True, stop=False)
                            # DoubleRow identity-add of bs: lhsT [I|0] or [0|I]
                            base = (c // 2) * 1024
                            idv = idDR_s[:, (c % 2) * 128:(c % 2) * 128 + 256]
                            nc.tensor.matmul(
                                pr_ps[:, c * 512:(c + 1) * 512],
                                idv.rearrange("p (j m) -> p j m", j=2, m=128),
                                bst[:, base:base + 1024].rearrange(
                                    "p (j n) -> p j n", j=2, n=512),
                                start=False, stop=True, perf_mode=DR)
                        nc.scalar.activation(Pc_s[:, nt * N:(nt + 1) * N],
                                             pr_ps[:], FN.Tanh, scale=0.5)

                # == S_pre = SV*Vs @ Pc (fp8 DoubleRow); E = exp(psum/(2SV)+rowb)
                # colsum via DVE accumulate + one GPSIMD partition all-reduce.
                pc_v = Pc_s[:].rearrange("p (j n) -> p j n", j=NTILES, n=N)
                with tc.tile_pool(name="vstr", bufs=4) as vstr, \
                     tc.tile_pool(name="accp", bufs=1) as accp, \
                     tc.tile_pool(name="sps", bufs=4, space="PSUM") as sps:
                    acc_s = accp.tile([128, N], F32, tag="acc")
                    cs_s = accp.tile([128, N], F32, tag="cs")
                    for mt in range(NTILES):
                        vt = vstr.tile([128, N], F8, tag="vt")
                        nc.sync.dma_start(vt[:], vst[mt, :, :])
                        vt_v = vt[:].rearrange("p (j m) -> p j m", j=NTILES, m=128)
                        for h in range(2):
                            s_ps = sps.tile([128, N // 2], F32, tag="s_ps")
                            for c in range(2):
                                col = h * 1024 + c * 512
                                for jp in range(NTILES // 2):
                                    nc.tensor.matmul(
                                        s_ps[:, c * 512:(c + 1) * 512],
                                        vt_v[:, 2 * jp:2 * jp + 2, :],
                                        pc_v[:, 2 * jp:2 * jp + 2, col:col + 512],
                                        start=(jp == 0),
                                        stop=(jp == NTILES // 2 - 1),
                                        perf_mode=DR)
                            ev = E_s[:, mt * N + h * 1024:mt * N + (h + 1) * 1024]
                            nc.scalar.activation(ev, s_ps[:], FN.Exp,
                                                 bias=rowb_s[:, mt:mt + 1],
                                                 scale=1.0 / (2.0 * SV))
                        if mt == 0:
                            nc.vector.tensor_copy(
                                acc_s[:], E_s[:, 0:N])
                        else:
                            nc.vector.tensor_add(
                                acc_s[:], acc_s[:], E_s[:, mt * N:(mt + 1) * N])
                    import concourse.bass_isa as bass_isa
                    nc.gpsimd.partition_all_reduce(
                        cs_s[:], acc_s[:], channels=128,
                        reduce_op=bass_isa.ReduceOp.add)
                    with nc.allow_low_precision(reason="colsum ~2048*E; bf16 "
                                                "rec only scales softmax"):
                        nc.vector.reciprocal(rec_s[:], cs_s[0:1, :])
                nc.gpsimd.partition_broadcast(rec_b[:], rec_s[:])

            # ========== cheb: Z_k = x_natt^T @ (cheb_k * E) ==========
            # k=0: cheb_0 == I, so only diagonal blocks of E contribute.
            nc.vector.memset(z_sb[:], 0.0)
            with tc.tile_pool(name="chstr", bufs=6) as chstr, \
                 tc.tile_pool(name="wtmp", bufs=6) as wtmp, \
                 tc.tile_pool(name="zps", bufs=2, space="PSUM") as zps:
                z0_ps = zps.tile([FT, N], F32, tag="z_ps")
                for mt in range(NTILES):
                    wt0 = wtmp.tile([128, 128], BF, tag="wt0")
                    d0 = mt * N + mt * 128
                    nc.vector.tensor_mul(wt0[:], E_s[:, d0:d0 + 128], idb_s[:])
                    nc.tensor.matmul(z0_ps[:, mt * 128:(mt + 1) * 128],
                                     xnatt_s[:, mt * FT:(mt + 1) * FT],
                                     wt0[:], start=True, stop=True)
                nc.vector.tensor_mul(z_sb[0:FT, :], z0_ps[:], rec_b[:])
                for k in range(1, K):
                    z_ps = zps.tile([FT, N], F32, tag="z_ps")
                    for mt in range(NTILES):
                        if k == 1 and mt < NPRE:
                            cht_ap = chpre[mt][:]
                        else:
                            cht = chstr.tile([128, N], BF, tag="cht")
                            nc.sync.dma_start(
                                cht[:], chebb[k - 1, mt * 128:(mt + 1) * 128, :])
                            cht_ap = cht[:]
                        wt = wtmp.tile([128, N], BF, tag="wt")
                        eng = nc.gpsimd if (mt % 4) == 3 else nc.vector
                        eng.tensor_mul(wt[:], cht_ap,
                                       E_s[:, mt * N:(mt + 1) * N])
                        for c in range(4):
                            nc.tensor.matmul(
                                z_ps[:, c * 512:(c + 1) * 512],
                                xnatt_s[:, mt * FT:(mt + 1) * FT],
                                wt[:, c * 512:(c + 1) * 512],
                                start=(mt == 0), stop=(mt == NTILES - 1))
                    nc.vector.tensor_mul(z_sb[32 * k:32 * k + FT, :], z_ps[:],
                                         rec_b[:])

        # ================= gcn =================
        with tc.tile_pool(name="gcnp", bufs=1) as gcnp, \
             tc.tile_pool(name="ypp", bufs=2) as ypp, \
             tc.tile_pool(name="ynp", bufs=4) as ynp, \
             tc.tile_pool(name="lns", bufs=4) as lns:
            # gcnB rows 0-63: gcn_pad blocks 0..13 ([0,g0..g11,0]); rows
            # 64-127: gcn_pad blocks shifted by one ([g0..g11,0,-]).
            gcnB = gcnp.tile([128, (T + 2) * N], BF, tag="gcnB")
            # gcnC rows 0-63: gcn_pad blocks shifted by two; rows 64-87: xtfn
            # replicated at every t block (residual-conv input).
            gcnC = gcnp.tile([88, T * N], BF, tag="gcnC")
            nc.vector.memset(gcnB[0:64, 0:N], 0.0)
            nc.vector.memset(gcnB[0:64, (T + 1) * N:(T + 2) * N], 0.0)
            for t in range(T):
                nc.sync.dma_start(gcnC[64:88, t * N:(t + 1) * N], xtfn_s[:])
            with tc.tile_pool(name="gps", bufs=2, space="PSUM") as gps:
                for t in range(T):
                    g_ps = gps.tile([O, N], F32, tag="g_ps")
                    for c in range(4):
                        nc.tensor.matmul(
                            g_ps[:, c * 512:(c + 1) * 512],
                            thg_s[:, t * O:(t + 1) * O],
                            z_sb[:, c * 512:(c + 1) * 512],
                            start=True, stop=True)
                    nc.scalar.activation(gcnB[0:64, (t + 1) * N:(t + 2) * N],
                                         g_ps[:], FN.Relu)
                    # shifted copies for the packed-contraction tconv
                    nc.sync.dma_start(gcnB[64:128, t * N:(t + 1) * N],
                                      gcnB[0:64, (t + 1) * N:(t + 2) * N])
                    if t >= 2:
                        nc.sync.dma_start(
                            gcnC[0:64, (t - 2) * N:(t - 1) * N],
                            gcnB[0:64, t * N:(t + 1) * N])
            for t in (T - 2, T - 1):
                nc.sync.dma_start(gcnC[0:64, t * N:(t + 1) * N],
                                  gcnB[0:64, (t + 2) * N:(t + 3) * N])

            # ====== tconv + res, with layernorm interleaved per column-chunk
            gfull_s = gcnp.tile([128, OT], BF, tag="gfull_s")
            nc.sync.dma_start(gfull_s[:], gfull[:])
            bfull_s = gcnp.tile([128, OT], BF, tag="bfull_s")
            nc.sync.dma_start(bfull_s[:], bfull[:])
            epsb = gcnp.tile([128, 1], F32, tag="epsb")
            nc.vector.memset(epsb[:], float(O) * LN_EPS)
            with tc.tile_pool(name="tps", bufs=6, space="PSUM") as tps, \
                 tc.tile_pool(name="stp", bufs=2, space="PSUM") as stp:
                for c in range(4):
                    ypre_s = ypp.tile([O, T * 512], BF, tag="ypre")
                    for t in range(T):
                        tc_ps = tps.tile([O, 512], F32, tag="tc_ps")
                        nc.tensor.matmul(
                            tc_ps[:], twa_s[:],
                            gcnB[:, t * N + c * 512:t * N + (c + 1) * 512],
                            start=True, stop=False)
                        nc.tensor.matmul(
                            tc_ps[:], twb_s[:, t * O:(t + 1) * O],
                            gcnC[:, t * N + c * 512:t * N + (c + 1) * 512],
                            start=False, stop=True)
                        nc.scalar.activation(
                            ypre_s[:, t * 512:(t + 1) * 512],
                            tc_ps[:], FN.Relu, bias=biaso_s[:], scale=1.0)
                    # per-(t,n) mean-sums on PE (after the t loop so the PE
                    # stream does not wait on each relu)
                    st_ps = stp.tile([T, 512], F32, tag="st_ps")
                    for t in range(T):
                        nc.tensor.matmul(
                            st_ps[:], etm_s[:, t * T:(t + 1) * T],
                            ypre_s[:, t * 512:(t + 1) * 512],
                            start=(t == 0), stop=(t == T - 1))
                    st_sb = lns.tile([T, 512], BF, tag="st_sb")
                    nc.vector.tensor_copy(st_sb[:], st_ps[:])
                    st_lo = lns.tile([T, 512], BF, tag="st_lo")
                    nc.vector.tensor_sub(st_lo[:], st_ps[:], st_sb[:])
                    ypdv = ypd_c[c][0:OT, :].rearrange("(t o) n -> o t n",
                                                       o=O, t=T)
                    nc.sync.dma_start(
                        ypdv[:, :, :],
                        ypre_s[:].rearrange("o (t n) -> o t n", t=T))
                    nc.sync.dma_start(ypd_c[c][OT:OT + T, :], st_sb[:])
                    nc.sync.dma_start(ypd_c[c][OT + T:OT + 2 * T, :], st_lo[:])
                    # ---- layernorm for the 4 n-tiles of this chunk ----
                    for q in range(4):
                        nt = c * 4 + q
                        yt = ynp.tile([128, YR], BF, tag="yt")
                        nc.sync.dma_start_transpose(
                            yt[:], ypd_c[c][:, q * 128:(q + 1) * 128])
                        ytv = yt[:, 0:OT].rearrange("p (t o) -> p t o",
                                                    o=O, t=T)
                        mus = lns.tile([128, T], F32, tag="mus")
                        nc.gpsimd.tensor_add(mus[:], yt[:, OT:OT + T],
                                             yt[:, OT + T:OT + 2 * T])
                        mud = lns.tile([128, T], F32, tag="mud")
                        nc.gpsimd.tensor_scalar_mul(mud[:], mus[:], 1.0 / O)
                        sqf = ynp.tile([128, OT], BF, tag="sqf")
                        seng = nc.vector if (q % 2) == 0 else nc.gpsimd
                        seng.tensor_mul(sqf[:], yt[:, 0:OT], yt[:, 0:OT])
                        sqs = lns.tile([128, T], F32, tag="sqs")
                        nc.vector.tensor_reduce(
                            sqs[:], sqf[:].rearrange("p (t o) -> p t o",
                                                     o=O, t=T),
                            axis=AX.X, op=OP.add)
                        w_t = lns.tile([128, T], F32, tag="w_t")
                        nc.gpsimd.tensor_mul(w_t[:], mus[:], mud[:])
                        nc.gpsimd.tensor_sub(w_t[:], sqs[:], w_t[:])  # 64*var
                        s_t = lns.tile([128, T], F32, tag="s_t")
                        nc.scalar.activation(s_t[:], w_t[:], FN.Sqrt,
                                             bias=epsb[:])
                        r_t = lns.tile([128, T], F32, tag="r_t")
                        nc.vector.reciprocal(r_t[:], s_t[:])  # isig/8
                        nmr = lns.tile([128, T], F32, tag="nmr")
                        nc.gpsimd.tensor_scalar_mul(nmr[:], mud[:], -1.0)
                        nc.gpsimd.tensor_mul(nmr[:], nmr[:], r_t[:])
                        yh = ynp.tile([128, OT], BF, tag="yh")
                        yhv = yh[:].rearrange("p (t o) -> p t o", o=O, t=T)
                        for t in range(T):
                            nc.scalar.activation(
                                yhv[:, t, :], ytv[:, t, :], FN.Identity,
                                bias=nmr[:, t:t + 1], scale=r_t[:, t:t + 1])
                        yg = ynp.tile([128, OT], BF, tag="yg")
                        nc.vector.tensor_mul(yg[:], yh[:], gfull_s[:])
                        aeng = nc.vector if (q % 2) == 0 else nc.gpsimd
                        aeng.tensor_add(yg[:], yg[:], bfull_s[:])
                        nc.sync.dma_start(y_d[nt * 128:(nt + 1) * 128, :],
                                          yg[:])
        es2.close()

        es.close()
    nc.compile()
    return nc


def _pack(x_b, consts):
    m = dict(consts)
    xb = np.asarray(x_b, np.float32)
    x_nat = xb.reshape(N, FT)
    x_natt = np.ascontiguousarray(xb.transpose(0, 2, 1)).reshape(N, FT)
    pk = lambda a: np.ascontiguousarray(
        a.reshape(NTILES, 128, FT).transpose(1, 0, 2).reshape(128, NTILES * FT)
    ).astype(bf16)
    m["xnat"] = pk(x_nat)
    m["xnatt"] = pk(x_natt)
    m["xftn"] = np.ascontiguousarray(xb.transpose(1, 2, 0).reshape(FT, N)).astype(bf16)
    m["xtfn"] = np.ascontiguousarray(xb.transpose(2, 1, 0).reshape(FT, N)).astype(bf16)
    return m


def kernel(**inputs):
    if "nc" not in _CACHE:
        _CACHE["nc"] = _build()
    nc = _CACHE["nc"]

    f32 = lambda a: np.asarray(a, np.float32)
    x = f32(inputs["x"])
    U1, U2, U3 = f32(inputs["U1"]), f32(inputs["U2"]), f32(inputs["U3"])
    Vs, bs = f32(inputs["Vs"]), f32(inputs["bs"])
    cheb = f32(inputs["cheb"])
    Theta = f32(inputs["Theta"])
    W1, W2, W3 = f32(inputs["W1"]), f32(inputs["W2"]), f32(inputs["W3"])
    be, Ve = f32(inputs["be"]), f32(inputs["Ve"])
    tw, tb = f32(inputs["tconv_w"]), f32(inputs["tconv_b"])
    rw, rb = f32(inputs["rconv_w"]), f32(inputs["rconv_b"])
    g, bb = f32(inputs["ln_g"]), f32(inputs["ln_b"])

    consts = {}
    consts["u1t"] = np.ascontiguousarray(U1.reshape(NTILES, 128).T).astype(bf16)
    consts["u2t"] = np.ascontiguousarray(
        U2.T.reshape(NTILES, 128, F).transpose(1, 0, 2).reshape(128, NTILES * F)
    ).astype(bf16)
    consts["u3b"] = np.broadcast_to(U3[None, :], (128, 2)).copy()
    consts["w1"] = W1[:, None].astype(bf16)
    consts["w2"] = W2.astype(bf16)
    consts["w3b"] = np.broadcast_to(W3[None, :], (128, 2)).copy()
    consts["bes"] = np.ascontiguousarray(be[0])
    consts["vet"] = np.ascontiguousarray(Ve.T)
    consts["ident"] = np.eye(T, dtype=np.float32)
    idDR = np.zeros((128, 384), np.float32)
    idDR[:, 0:128] = np.eye(128)
    idDR[:, 256:384] = np.eye(128)
    consts["idDR"] = idDR.astype(f8e4)
    consts["idbf"] = np.eye(128, dtype=np.float32).astype(bf16)
    consts["bsb"] = bs[0].astype(f8e4)
    VsT = np.ascontiguousarray(Vs.T)  # [n, m]
    consts["vst"] = np.ascontiguousarray(
        (SV * VsT).reshape(NTILES, 128, NTILES, 128).transpose(2, 1, 0, 3)
        .reshape(NTILES, 128, N)
    ).astype(f8e4)
    consts["rowb"] = np.ascontiguousarray(
        (0.5 * Vs.sum(axis=1)).reshape(NTILES, 128).T).astype(np.float32)
    consts["chebb"] = cheb[1:].astype(bf16)
    thg = np.zeros((96, T * O), np.float32)
    for t in range(T):
        for k in range(K):
            for f in range(F):
                thg[32 * k + t * F + f, t * O:(t + 1) * O] = Theta[k, f]
    consts["thg"] = thg.astype(bf16)
    # twa: [tw_dt0 ; tw_dt1] stacked along contraction; twb: per-t
    # [tw_dt2 (64 rows) ; residual-conv rows (24)]
    twl = tw[:, :, 0, :].transpose(1, 2, 0).reshape(O, 3 * O)  # [i, (dt, o)]
    consts["twa"] = np.concatenate(
        [twl[:, 0:O], twl[:, O:2 * O]], axis=0).astype(bf16)
    twb = np.zeros((88, T * O), np.float32)
    for t in range(T):
        twb[0:O, t * O:(t + 1) * O] = twl[:, 2 * O:3 * O]
        for f in range(F):
            twb[O + t * F + f, t * O:(t + 1) * O] = rw[:, f, 0, 0]
    consts["twb"] = twb.astype(bf16)
    etm_np = np.zeros((O, T * T), np.float32)
    for t in range(T):
        etm_np[:, t * T + t] = 1.0
    consts["etm"] = etm_np.astype(bf16)
    consts["biaso"] = (tb + rb)[:, None]
    consts["gfull"] = np.broadcast_to(
        (8.0 * np.tile(g, T))[None, :], (128, OT)).astype(bf16).copy()
    consts["bfull"] = np.broadcast_to(
        np.tile(bb, T)[None, :], (128, OT)).astype(bf16).copy()
    consts["onesb"] = np.ones((128, 1), bf16)

    in_maps = [_pack(x[b], consts) for b in range(B)]
    _CACHE["im"] = in_maps
    res = run_bass_kernel_spmd(nc, in_maps, list(range(B)))
    out = np.stack([
        np.asarray(res.results[b]["y"], np.float32)
        .reshape(N, T, O).transpose(0, 2, 1)
        for b in range(B)
    ])
    return out.astype(np.float32)


# revision 52
# speedup vs baseline: 1.0880x; 1.0837x over previous
import sys
if '/opt/trn_rl_repo' not in sys.path:
    sys.path.insert(0, '/opt/trn_rl_repo')
import numpy as np
import ml_dtypes

import concourse.bass as bass
import concourse.bacc as bacc
import concourse.tile as tile
from concourse import mybir
from concourse.bass_utils import run_bass_kernel_spmd

BF = mybir.dt.bfloat16
F32 = mybir.dt.float32
F8 = mybir.dt.float8e4
FN = mybir.ActivationFunctionType
OP = mybir.AluOpType
AX = mybir.AxisListType
DR = mybir.MatmulPerfMode.DoubleRow

B, N, F, T = 8, 2048, 2, 12
K, O = 3, 64
FT = F * T           # 24
NTILES = N // 128    # 16
OT = O * T           # 768
LN_EPS = 1e-5
SV = 8.0             # scale folded into fp8 Vs (S_pre = psum/(2*SV) + rowb)

bf16 = ml_dtypes.bfloat16
f8e4 = ml_dtypes.float8_e4m3

_CACHE = {}


def _build():
    nc = bacc.Bacc("TRN2", target_bir_lowering=False, debug=False, num_devices=8)

    def din(name, shape, dt=BF):
        return nc.declare_dram_parameter(name, list(shape), dt, isOutput=False)

    xnat = din("xnat", (128, NTILES * FT))        # x[b] [n,(f,t)] tiled: part=n%128
    xnatt = din("xnatt", (128, NTILES * FT))      # x[b] [n,(t,f)] tiled
    xftn = din("xftn", (FT, N))                   # [(f,t), n]
    xtfn = din("xtfn", (FT, N))                   # [(t,f), n]
    u1t = din("u1t", (128, NTILES))
    u2t = din("u2t", (128, NTILES * F))
    u3b = din("u3b", (128, 2), F32)
    w1 = din("w1", (T, 1))
    w2 = din("w2", (F, T))
    w3b = din("w3b", (128, 2), F32)
    bes = din("bes", (T, T), F32)
    vet = din("vet", (T, T), F32)
    ident = din("ident", (T, T), F32)
    bsb = din("bsb", (N, N), F8)                  # fp8 spatial-attn bias
    idDR = din("idDR", (128, 384), F8)            # [I|0|I] for DoubleRow bias-add
    idbf = din("idbf", (128, 128))                # identity bf16 (diag mask)
    vst = din("vst", (NTILES, 128, N), F8)        # per m-tile: SV*Vs^T fp8
    rowb = din("rowb", (128, NTILES), F32)        # 0.5*rowsum(Vs) per m
    chebb = din("chebb", (K - 1, N, N))           # cheb k=1,2 only (k=0 == I)
    thg = din("thg", (96, T * O))                 # zero-padded gcn lhsT per t
    twa = din("twa", (128, O))                    # [dt0|dt1] stacked tconv lhsT
    twb = din("twb", (88, T * O))                 # [dt2 ; res] per-t tconv lhsT
    biaso = din("biaso", (O, 1), F32)
    gfull = din("gfull", (128, OT))               # ln_g in (t,o) order, bf16
    bfull = din("bfull", (128, OT))               # ln_b in (t,o) order, bf16
    onesb = din("onesb", (128, 1))

    y_d = nc.declare_dram_parameter("y", [N, OT], BF, isOutput=True)  # (t,o) order
    # per-512-column scratch chunks [(t,o), 512] for exact LN pipelining deps
    ypd_c = [nc.dram_tensor("ypd%d" % c, [OT, 512], BF) for c in range(4)]

    with tile.TileContext(nc) as tc:
        from contextlib import ExitStack
        es = ExitStack()
        sm = es.enter_context(tc.tile_pool(name="sm", bufs=1))

        def load(dram, shape, dt=BF):
            t_ = sm.tile(list(shape), dt, tag=dram.name + "_s")
            nc.sync.dma_start(t_[:], dram[:])
            return t_

        xnat_s = load(xnat, (128, NTILES * FT))
        xnatt_s = load(xnatt, (128, NTILES * FT))

        xtfn_s = load(xtfn, (FT, N))
        u1t_s = load(u1t, (128, NTILES))
        u2t_s = load(u2t, (128, NTILES * F))
        u3b_s = load(u3b, (128, 2), F32)
        w1_s = load(w1, (T, 1))
        w2_s = load(w2, (F, T))
        w3b_s = load(w3b, (128, 2), F32)
        bes_s = load(bes, (T, T), F32)
        vet_s = load(vet, (T, T), F32)
        id_s = load(ident, (T, T), F32)
        idDR_s = load(idDR, (128, 384), F8)
        idb_s = load(idbf, (128, 128))
        rowb_s = load(rowb, (128, NTILES), F32)
        thg_s = load(thg, (96, T * O))
        twa_s = load(twa, (128, O))
        twb_s = load(twb, (88, T * O))
        biaso_s = load(biaso, (O, 1), F32)
        ones_s = load(onesb, (128, 1))

        # ================= temporal attention =================
        rhss_s = sm.tile([T, N], BF, tag="rhss")
        l2t_s = sm.tile([T, N], BF, tag="l2t")
        with tc.tile_pool(name="pst", bufs=1, space="PSUM") as pst, \
             tc.tile_pool(name="pat", bufs=1, space="PSUM") as pat, \
             tc.tile_pool(name="attp", bufs=1) as attp:
            xftn_f = []
            for f in range(F):
                t_ = attp.tile([T, N], BF, tag="xftn%d" % f)
                nc.sync.dma_start(t_[:], xftn[f * T:(f + 1) * T, :])
                xftn_f.append(t_)
            # xU1 [1, 24] = sum_n U1[n] * x_nat[n, (f,t)]
            xu1_ps = pst.tile([1, FT], F32, tag="tiny_ps")
            for j in range(NTILES):
                nc.tensor.matmul(xu1_ps[:], u1t_s[:, j:j + 1],
                                 xnat_s[:, j * FT:(j + 1) * FT],
                                 start=(j == 0), stop=(j == NTILES - 1))
            xu1_ft = sm.tile([F, T], F32, tag="xu1ft")
            xu1_row = attp.tile([1, FT], F32, tag="xu1row")
            nc.vector.tensor_copy(xu1_row[:], xu1_ps[:])
            for f in range(F):
                nc.sync.dma_start(xu1_ft[f:f + 1, :],
                                  xu1_row[0:1, f * T:(f + 1) * T])

            # rhs_t [n, u] (packed [128, (j,u)]) = x[.,0,u]*U3[0] + x[.,1,u]*U3[1]
            rhs_t = attp.tile([128, NTILES * T], BF, tag="rhs_t")
            xf0 = xnat_s[:].rearrange("p (j f t) -> p j f t", j=NTILES, f=F, t=T)
            tmp_rt = attp.tile([128, NTILES * T], BF, tag="tmp_rt")
            rt_v = tmp_rt[:].rearrange("p (j t) -> p j t", j=NTILES, t=T)
            rhs_tv = rhs_t[:].rearrange("p (j t) -> p j t", j=NTILES, t=T)
            nc.vector.tensor_scalar_mul(rt_v, xf0[:, :, 1, :], u3b_s[:, 1:2])
            nc.vector.scalar_tensor_tensor(rhs_tv, xf0[:, :, 0, :], u3b_s[:, 0:1],
                                           rt_v, op0=OP.mult, op1=OP.add)

            # M [f, u] = sum_n U2^T-tiles @ rhs_t
            m_ps = pst.tile([F, T], F32, tag="tiny_ps")
            for j in range(NTILES):
                nc.tensor.matmul(m_ps[:], u2t_s[:, j * F:(j + 1) * F],
                                 rhs_t[:, j * T:(j + 1) * T],
                                 start=(j == 0), stop=(j == NTILES - 1))
            m_s = sm.tile([F, T], F32, tag="m_s")
            nc.vector.tensor_copy(m_s[:], m_ps[:])

            # prod_t [t, u] = xu1_ft^T @ M ; PT = sigmoid(prod_t + be)
            pt_ps = pst.tile([T, T], F32, tag="tiny_ps")
            nc.tensor.matmul(pt_ps[:], xu1_ft[:], m_s[:], start=True, stop=True)
            pt_sb = sm.tile([T, T], F32, tag="pt_sb")
            nc.vector.tensor_add(pt_sb[:], pt_ps[:], bes_s[:])
            nc.scalar.activation(pt_sb[:], pt_sb[:], FN.Sigmoid)

            # E0 [t, v] = Ve @ PT  (lhsT = Ve^T)
            e0_ps = pst.tile([T, T], F32, tag="tiny_ps")
            nc.tensor.matmul(e0_ps[:], vet_s[:], pt_sb[:], start=True, stop=True)
            e0_s = sm.tile([T, T], F32, tag="e0_s")
            nc.vector.tensor_copy(e0_s[:], e0_ps[:])
            e0t_ps = pst.tile([T, T], F32, tag="tiny_ps")
            nc.tensor.transpose(e0t_ps[:], e0_s[:], id_s[:])
            e0t_s = sm.tile([T, T], F32, tag="e0t_s")
            nc.vector.tensor_copy(e0t_s[:], e0t_ps[:])
            # softmax along free (t) for each v
            mx = sm.tile([T, 1], F32, tag="mx")
            nc.vector.tensor_reduce(mx[:], e0t_s[:], axis=AX.X, op=OP.max)
            nmx = sm.tile([T, 1], F32, tag="nmx")
            nc.vector.tensor_scalar_mul(nmx[:], mx[:], -1.0)
            esum = sm.tile([T, 1], F32, tag="esum")
            nc.scalar.activation(e0t_s[:], e0t_s[:], FN.Exp, bias=nmx[:],
                                 scale=1.0, accum_out=esum[:])
            recs = sm.tile([T, 1], F32, tag="recs")
            nc.vector.reciprocal(recs[:], esum[:])
            nc.vector.tensor_scalar_mul(e0t_s[:], e0t_s[:], recs[:])
            esm_ps = pst.tile([T, T], F32, tag="tiny_ps")
            nc.tensor.transpose(esm_ps[:], e0t_s[:], id_s[:])
            esm_bf = sm.tile([T, T], BF, tag="esm_bf")
            nc.vector.tensor_copy(esm_bf[:], esm_ps[:])

            # xTA_ftn [(f,t), n] : per f: Esm^T @ x_ftn[f]
            xta_f = []
            for f in range(F):
                xta_t = attp.tile([T, N], BF, tag="xta%d" % f)
                xta_ps = pat.tile([T, N], F32, tag="attnps")
                for c in range(4):
                    nc.tensor.matmul(xta_ps[:, c * 512:(c + 1) * 512], esm_bf[:],
                                     xftn_f[f][:, c * 512:(c + 1) * 512],
                                     start=True, stop=True)
                nc.vector.tensor_copy(xta_t[:], xta_ps[:])
                xta_f.append(xta_t)

            # ============== spatial attention (small parts) ==============
            tmp_rs = attp.tile([T, N], BF, tag="tmp_rs")
            nc.vector.tensor_scalar_mul(tmp_rs[:], xta_f[1][:],
                                        w3b_s[0:T, 1:2])
            nc.vector.scalar_tensor_tensor(rhss_s[:], xta_f[0][:],
                                           w3b_s[0:T, 0:1], tmp_rs[:],
                                           op0=OP.mult, op1=OP.add)
            lsfn_s = sm.tile([F, N], BF, tag="lsfn")
            for f in range(F):
                ls_ps = pat.tile([1, N], F32, tag="attnps")
                for c in range(4):
                    nc.tensor.matmul(ls_ps[:, c * 512:(c + 1) * 512], w1_s[:],
                                     xta_f[f][:, c * 512:(c + 1) * 512],
                                     start=True, stop=True)
                if f == 0:
                    nc.vector.tensor_copy(lsfn_s[0:1, :], ls_ps[:])
                else:
                    ls1 = attp.tile([1, N], BF, tag="ls1")
                    nc.vector.tensor_copy(ls1[:], ls_ps[:])
                    nc.sync.dma_start(lsfn_s[1:2, :], ls1[:])
            l2_ps = pat.tile([T, N], F32, tag="attnps")
            for c in range(4):
                nc.tensor.matmul(l2_ps[:, c * 512:(c + 1) * 512], w2_s[:],
                                 lsfn_s[:, c * 512:(c + 1) * 512],
                                 start=True, stop=True)
            nc.vector.tensor_copy(l2t_s[:], l2_ps[:])

        # ========== Pc = tanh(0.5*(prod_s + bs)) in fp8 ==========
        from contextlib import ExitStack as _ES
        es2 = _ES()
        midp = es2.enter_context(tc.tile_pool(name="midp", bufs=1))
        rec_s = midp.tile([1, N], BF, tag="rec_s")
        rec_b = midp.tile([FT, N], BF, tag="rec_b")
        z_sb = midp.tile([96, N], BF, tag="z_sb")
        NPRE = 8  # cheb tiles prefetched into SBUF during the P phase
        with tc.tile_pool(name="Epool", bufs=1) as epool:
            E_s = epool.tile([128, NTILES * N], BF, tag="E")
            chpre = [epool.tile([128, N], BF, tag="chpre%d" % i,
                                name="chpre%d" % i)
                     for i in range(NPRE)]
            with tc.tile_pool(name="Ppool", bufs=1) as ppool:
                Pc_s = ppool.tile([128, NTILES * N], F8, tag="Pc")
                with tc.tile_pool(name="bsstr", bufs=3) as bsstr, \
                     tc.tile_pool(name="pps", bufs=2, space="PSUM") as pps:
                    for nt in range(NTILES):
                        bst = bsstr.tile([128, N], F8, tag="bst")
                        nc.sync.dma_start(bst[:], bsb[nt * 128:(nt + 1) * 128, :])
                        if nt < NPRE:  # prefetch cheb k=1 while DMA is idle
                            nc.sync.dma_start(
                                chpre[nt][:], chebb[0, nt * 128:(nt + 1) * 128, :])
                        pr_ps = pps.tile([128, N], F32, tag="pr_ps")
                        for c in range(4):
                            nc.tensor.matmul(pr_ps[:, c * 512:(c + 1) * 512],
                                             l2t_s[:, nt * 128:(nt + 1) * 128],
                                             rhss_s[:, c * 512:(c + 1) * 512],
                                             start=True, stop=False)
                            # DoubleRow identity-add of bs: lhsT [I|0]/[0|I]
                            base = (c // 2) * 1024
                            idv = idDR_s[:, (c % 2) * 128:(c % 2) * 128 + 256]
                            nc.tensor.matmul(
                                pr_ps[:, c * 512:(c + 1) * 512],
                                idv.rearrange("p (j m) -> p j m", j=2, m=128),
                                bst[:, base:base + 1024].rearrange(
                                    "p (j n) -> p j n", j=2, n=512),
                                start=False, stop=True, perf_mode=DR)
                        nc.scalar.activation(Pc_s[:, nt * N:(nt + 1) * N],
                                             pr_ps[:], FN.Tanh, scale=0.5)

                # == S_pre = SV*Vs @ Pc (fp8 DoubleRow); E = exp(psum/(2SV)+rowb)
                # colsum via DVE accumulate + one GPSIMD partition all-reduce.
                pc_v = Pc_s[:].rearrange("p (j n) -> p j n", j=NTILES, n=N)
                with tc.tile_pool(name="vstr", bufs=4) as vstr, \
                     tc.tile_pool(name="accp", bufs=1) as accp, \
                     tc.tile_pool(name="sps", bufs=2, space="PSUM") as sps:
                    acc_s = accp.tile([128, N], F32, tag="acc")
                    cs_s = accp.tile([128, N], F32, tag="cs")
                    for mt in range(NTILES):
                        vt = vstr.tile([128, N], F8, tag="vt")
                        nc.sync.dma_start(vt[:], vst[mt, :, :])
                        vt_v = vt[:].rearrange("p (j m) -> p j m", j=NTILES, m=128)
                        s_ps = sps.tile([128, N], F32, tag="s_ps")
                        for c in range(4):
                            for jp in range(NTILES // 2):
                                nc.tensor.matmul(
                                    s_ps[:, c * 512:(c + 1) * 512],
                                    vt_v[:, 2 * jp:2 * jp + 2, :],
                                    pc_v[:, 2 * jp:2 * jp + 2,
                                         c * 512:(c + 1) * 512],
                                    start=(jp == 0),
                                    stop=(jp == NTILES // 2 - 1),
                                    perf_mode=DR)
                        ev = E_s[:, mt * N:(mt + 1) * N]
                        nc.scalar.activation(ev, s_ps[:], FN.Exp,
                                             bias=rowb_s[:, mt:mt + 1],
                                             scale=1.0 / (2.0 * SV))
                        if mt == 0:
                            nc.vector.tensor_copy(
                                acc_s[:], E_s[:, 0:N])
                        else:
                            nc.vector.tensor_add(
                                acc_s[:], acc_s[:], E_s[:, mt * N:(mt + 1) * N])
                    import concourse.bass_isa as bass_isa
                    nc.gpsimd.partition_all_reduce(
                        cs_s[:], acc_s[:], channels=128,
                        reduce_op=bass_isa.ReduceOp.add)
                    with nc.allow_low_precision(reason="colsum ~2048*E; bf16 "
                                                "rec only scales softmax"):
                        nc.vector.reciprocal(rec_s[:], cs_s[0:1, :])
                nc.gpsimd.partition_broadcast(rec_b[:], rec_s[:])

            # ========== cheb: Z_k = x_natt^T @ (cheb_k * E) ==========
            # k=0: cheb_0 == I, so only diagonal blocks of E contribute.
            nc.vector.memset(z_sb[:], 0.0)
            with tc.tile_pool(name="chstr", bufs=6) as chstr, \
                 tc.tile_pool(name="wtmp", bufs=6) as wtmp, \
                 tc.tile_pool(name="zps", bufs=2, space="PSUM") as zps:
                z0_ps = zps.tile([FT, N], F32, tag="z_ps")
                for mt in range(NTILES):
                    wt0 = wtmp.tile([128, 128], BF, tag="wt0")
                    d0 = mt * N + mt * 128
                    nc.vector.tensor_mul(wt0[:], E_s[:, d0:d0 + 128], idb_s[:])
                    nc.tensor.matmul(z0_ps[:, mt * 128:(mt + 1) * 128],
                                     xnatt_s[:, mt * FT:(mt + 1) * FT],
                                     wt0[:], start=True, stop=True)
                nc.vector.tensor_mul(z_sb[0:FT, :], z0_ps[:], rec_b[:])
                for k in range(1, K):
                    z_ps = zps.tile([FT, N], F32, tag="z_ps")
                    for mt in range(NTILES):
                        if k == 1 and mt < NPRE:
                            cht_ap = chpre[mt][:]
                        else:
                            cht = chstr.tile([128, N], BF, tag="cht")
                            nc.sync.dma_start(
                                cht[:], chebb[k - 1, mt * 128:(mt + 1) * 128, :])
                            cht_ap = cht[:]
                        wt = wtmp.tile([128, N], BF, tag="wt")
                        eng = nc.gpsimd if (mt % 4) == 3 else nc.vector
                        eng.tensor_mul(wt[:], cht_ap,
                                       E_s[:, mt * N:(mt + 1) * N])
                        for c in range(4):
                            nc.tensor.matmul(
                                z_ps[:, c * 512:(c + 1) * 512],
                                xnatt_s[:, mt * FT:(mt + 1) * FT],
                                wt[:, c * 512:(c + 1) * 512],
                                start=(mt == 0), stop=(mt == NTILES - 1))
                    nc.vector.tensor_mul(z_sb[32 * k:32 * k + FT, :], z_ps[:],
                                         rec_b[:])

        # ================= gcn =================
        with tc.tile_pool(name="gcnp", bufs=1) as gcnp, \
             tc.tile_pool(name="ypp", bufs=2) as ypp, \
             tc.tile_pool(name="ynp", bufs=4) as ynp, \
             tc.tile_pool(name="lns", bufs=4) as lns:
            # gcnB rows 0-63: gcn_pad blocks 0..13 ([0,g0..g11,0]); rows
            # 64-127: gcn_pad blocks shifted by one ([g0..g11,0,-]).
            gcnB = gcnp.tile([128, (T + 2) * N], BF, tag="gcnB")
            # gcnC rows 0-63: gcn_pad blocks shifted by two; rows 64-87: xtfn
            # replicated at every t block (residual-conv input).
            gcnC = gcnp.tile([88, T * N], BF, tag="gcnC")
            nc.vector.memset(gcnB[0:64, 0:N], 0.0)
            nc.vector.memset(gcnB[0:64, (T + 1) * N:(T + 2) * N], 0.0)
            for t in range(T):
                nc.sync.dma_start(gcnC[64:88, t * N:(t + 1) * N], xtfn_s[:])
            with tc.tile_pool(name="gps", bufs=2, space="PSUM") as gps:
                for t in range(T):
                    g_ps = gps.tile([O, N], F32, tag="g_ps")
                    for c in range(4):
                        nc.tensor.matmul(
                            g_ps[:, c * 512:(c + 1) * 512],
                            thg_s[:, t * O:(t + 1) * O],
                            z_sb[:, c * 512:(c + 1) * 512],
                            start=True, stop=True)
                    nc.scalar.activation(gcnB[0:64, (t + 1) * N:(t + 2) * N],
                                         g_ps[:], FN.Relu)
                    # shifted copies for the packed-contraction tconv
                    nc.sync.dma_start(gcnB[64:128, t * N:(t + 1) * N],
                                      gcnB[0:64, (t + 1) * N:(t + 2) * N])
                    if t >= 2:
                        nc.sync.dma_start(
                            gcnC[0:64, (t - 2) * N:(t - 1) * N],
                            gcnB[0:64, t * N:(t + 1) * N])
            for t in (T - 2, T - 1):
                nc.sync.dma_start(gcnC[0:64, t * N:(t + 1) * N],
                                  gcnB[0:64, (t + 2) * N:(t + 3) * N])

            # ====== tconv + res, with layernorm interleaved per column-chunk
            gfull_s = gcnp.tile([128, OT], BF, tag="gfull_s")
            nc.sync.dma_start(gfull_s[:], gfull[:])
            bfull_s = gcnp.tile([128, OT], BF, tag="bfull_s")
            nc.sync.dma_start(bfull_s[:], bfull[:])
            epsb = gcnp.tile([128, 1], F32, tag="epsb")
            nc.vector.memset(epsb[:], float(O) * LN_EPS)
            with tc.tile_pool(name="tps", bufs=8, space="PSUM") as tps:
                for c in range(4):
                    ypre_s = ypp.tile([O, T * 512], BF, tag="ypre")
                    for t in range(T):
                        tc_ps = tps.tile([O, 512], F32, tag="tc_ps")
                        nc.tensor.matmul(
                            tc_ps[:], twa_s[:],
                            gcnB[:, t * N + c * 512:t * N + (c + 1) * 512],
                            start=True, stop=False)
                        nc.tensor.matmul(
                            tc_ps[:], twb_s[:, t * O:(t + 1) * O],
                            gcnC[:, t * N + c * 512:t * N + (c + 1) * 512],
                            start=False, stop=True)
                        nc.scalar.activation(
                            ypre_s[:, t * 512:(t + 1) * 512],
                            tc_ps[:], FN.Relu, bias=biaso_s[:], scale=1.0)
                    ypdv = ypd_c[c][:].rearrange("(t o) n -> o t n", o=O, t=T)
                    nc.sync.dma_start(
                        ypdv[:, :, :],
                        ypre_s[:].rearrange("o (t n) -> o t n", t=T))
                    # ---- layernorm for the 4 n-tiles of this chunk ----
                    for q in range(4):
                        nt = c * 4 + q
                        yt = ynp.tile([128, OT], BF, tag="yt")
                        nc.sync.dma_start_transpose(
                            yt[:], ypd_c[c][:, q * 128:(q + 1) * 128])
                        ytv = yt[:].rearrange("p (t o) -> p t o", o=O, t=T)
                        mus = lns.tile([128, T], F32, tag="mus")
                        nc.vector.tensor_reduce(mus[:], ytv, axis=AX.X,
                                                op=OP.add)
                        mud = lns.tile([128, T], F32, tag="mud")
                        nc.gpsimd.tensor_scalar_mul(mud[:], mus[:], 1.0 / O)
                        sqf = ynp.tile([128, OT], BF, tag="sqf")
                        seng = nc.vector if (q % 2) == 0 else nc.gpsimd
                        seng.tensor_mul(sqf[:], yt[:], yt[:])
                        sqs = lns.tile([128, T], F32, tag="sqs")
                        nc.vector.tensor_reduce(
                            sqs[:], sqf[:].rearrange("p (t o) -> p t o",
                                                     o=O, t=T),
                            axis=AX.X, op=OP.add)
                        w_t = lns.tile([128, T], F32, tag="w_t")
                        nc.gpsimd.tensor_mul(w_t[:], mus[:], mud[:])
                        nc.gpsimd.tensor_sub(w_t[:], sqs[:], w_t[:])  # 64*var
                        s_t = lns.tile([128, T], F32, tag="s_t")
                        nc.scalar.activation(s_t[:], w_t[:], FN.Sqrt,
                                             bias=epsb[:])
                        r_t = lns.tile([128, T], F32, tag="r_t")
                        nc.vector.reciprocal(r_t[:], s_t[:])  # isig/8
                        nmr = lns.tile([128, T], F32, tag="nmr")
                        nc.gpsimd.tensor_scalar_mul(nmr[:], mud[:], -1.0)
                        nc.gpsimd.tensor_mul(nmr[:], nmr[:], r_t[:])
                        yh = ynp.tile([128, OT], BF, tag="yh")
                        yhv = yh[:].rearrange("p (t o) -> p t o", o=O, t=T)
                        for t in range(T):
                            if t % 3 == 2:
                                nc.scalar.activation(
                                    yhv[:, t, :], ytv[:, t, :], FN.Identity,
                                    bias=nmr[:, t:t + 1], scale=r_t[:, t:t + 1])
                            else:
                                nc.vector.tensor_scalar(
                                    yhv[:, t, :], ytv[:, t, :],
                                    mud[:, t:t + 1], r_t[:, t:t + 1],
                                    op0=OP.subtract, op1=OP.mult)
                        yg = ynp.tile([128, OT], BF, tag="yg")
                        nc.vector.tensor_mul(yg[:], yh[:], gfull_s[:])
                        aeng = nc.vector if (q % 2) == 0 else nc.gpsimd
                        aeng.tensor_add(yg[:], yg[:], bfull_s[:])
                        nc.sync.dma_start(y_d[nt * 128:(nt + 1) * 128, :],
                                          yg[:])
        es2.close()

        es.close()
    nc.compile()
    return nc


def _pack(x_b, consts):
    m = dict(consts)
    xb = np.asarray(x_b, np.float32)
    x_nat = xb.reshape(N, FT)
    x_natt = np.ascontiguousarray(xb.transpose(0, 2, 1)).reshape(N, FT)
    pk = lambda a: np.ascontiguousarray(
        a.reshape(NTILES, 128, FT).transpose(1, 0, 2).reshape(128, NTILES * FT)
    ).astype(bf16)
    m["xnat"] = pk(x_nat)
    m["xnatt"] = pk(x_natt)
    m["xftn"] = np.ascontiguousarray(xb.transpose(1, 2, 0).reshape(FT, N)).astype(bf16)
    m["xtfn"] = np.ascontiguousarray(xb.transpose(2, 1, 0).reshape(FT, N)).astype(bf16)
    return m


def kernel(**inputs):
    if "nc" not in _CACHE:
        _CACHE["nc"] = _build()
    nc = _CACHE["nc"]

    f32 = lambda a: np.asarray(a, np.float32)
    x = f32(inputs["x"])
    U1, U2, U3 = f32(inputs["U1"]), f32(inputs["U2"]), f32(inputs["U3"])
    Vs, bs = f32(inputs["Vs"]), f32(inputs["bs"])
    cheb = f32(inputs["cheb"])
    Theta = f32(inputs["Theta"])
    W1, W2, W3 = f32(inputs["W1"]), f32(inputs["W2"]), f32(inputs["W3"])
    be, Ve = f32(inputs["be"]), f32(inputs["Ve"])
    tw, tb = f32(inputs["tconv_w"]), f32(inputs["tconv_b"])
    rw, rb = f32(inputs["rconv_w"]), f32(inputs["rconv_b"])
    g, bb = f32(inputs["ln_g"]), f32(inputs["ln_b"])

    consts = {}
    consts["u1t"] = np.ascontiguousarray(U1.reshape(NTILES, 128).T).astype(bf16)
    consts["u2t"] = np.ascontiguousarray(
        U2.T.reshape(NTILES, 128, F).transpose(1, 0, 2).reshape(128, NTILES * F)
    ).astype(bf16)
    consts["u3b"] = np.broadcast_to(U3[None, :], (128, 2)).copy()
    consts["w1"] = W1[:, None].astype(bf16)
    consts["w2"] = W2.astype(bf16)
    consts["w3b"] = np.broadcast_to(W3[None, :], (128, 2)).copy()
    consts["bes"] = np.ascontiguousarray(be[0])
    consts["vet"] = np.ascontiguousarray(Ve.T)
    consts["ident"] = np.eye(T, dtype=np.float32)
    idDR = np.zeros((128, 384), np.float32)
    idDR[:, 0:128] = np.eye(128)
    idDR[:, 256:384] = np.eye(128)
    consts["idDR"] = idDR.astype(f8e4)
    consts["idbf"] = np.eye(128, dtype=np.float32).astype(bf16)
    consts["bsb"] = bs[0].astype(f8e4)
    VsT = np.ascontiguousarray(Vs.T)  # [n, m]
    consts["vst"] = np.ascontiguousarray(
        (SV * VsT).reshape(NTILES, 128, NTILES, 128).transpose(2, 1, 0, 3)
        .reshape(NTILES, 128, N)
    ).astype(f8e4)
    consts["rowb"] = np.ascontiguousarray(
        (0.5 * Vs.sum(axis=1)).reshape(NTILES, 128).T).astype(np.float32)
    consts["chebb"] = cheb[1:].astype(bf16)
    thg = np.zeros((96, T * O), np.float32)
    for t in range(T):
        for k in range(K):
            for f in range(F):
                thg[32 * k + t * F + f, t * O:(t + 1) * O] = Theta[k, f]
    consts["thg"] = thg.astype(bf16)
    # twa: [tw_dt0 ; tw_dt1] stacked along contraction; twb: per-t
    # [tw_dt2 (64 rows) ; residual-conv rows (24)]
    twl = tw[:, :, 0, :].transpose(1, 2, 0).reshape(O, 3 * O)  # [i, (dt, o)]
    consts["twa"] = np.concatenate(
        [twl[:, 0:O], twl[:, O:2 * O]], axis=0).astype(bf16)
    twb = np.zeros((88, T * O), np.float32)
    for t in range(T):
        twb[0:O, t * O:(t + 1) * O] = twl[:, 2 * O:3 * O]
        for f in range(F):
            twb[O + t * F + f, t * O:(t + 1) * O] = rw[:, f, 0, 0]
    consts["twb"] = twb.astype(bf16)

    consts["biaso"] = (tb + rb)[:, None]
    consts["gfull"] = np.broadcast_to(
        (8.0 * np.tile(g, T))[None, :], (128, OT)).astype(bf16).copy()
    consts["bfull"] = np.broadcast_to(
        np.tile(bb, T)[None, :], (128, OT)).astype(bf16).copy()
    consts["onesb"] = np.ones((128, 1), bf16)

    in_maps = [_pack(x[b], consts) for b in range(B)]
    _CACHE["im"] = in_maps
    res = run_bass_kernel_spmd(nc, in_maps, list(range(B)))
    out = np.stack([
        np.asarray(res.results[b]["y"], np.float32)
        .reshape(N, T, O).transpose(0, 2, 1)
        for b in range(B)
    ])
    return out.astype(np.float32)


# revision 60
# speedup vs baseline: 1.1069x; 1.0174x over previous
import sys
if '/opt/trn_rl_repo' not in sys.path:
    sys.path.insert(0, '/opt/trn_rl_repo')
import numpy as np
import ml_dtypes

import concourse.bass as bass
import concourse.bacc as bacc
import concourse.tile as tile
from concourse import mybir
from concourse.bass_utils import run_bass_kernel_spmd

BF = mybir.dt.bfloat16
F32 = mybir.dt.float32
F8 = mybir.dt.float8e4
FN = mybir.ActivationFunctionType
OP = mybir.AluOpType
AX = mybir.AxisListType
DR = mybir.MatmulPerfMode.DoubleRow

B, N, F, T = 8, 2048, 2, 12
K, O = 3, 64
FT = F * T           # 24
NTILES = N // 128    # 16
OT = O * T           # 768
LN_EPS = 1e-5
SV = 8.0             # scale folded into fp8 Vs (S_pre = psum/(2*SV) + rowb)

bf16 = ml_dtypes.bfloat16
f8e4 = ml_dtypes.float8_e4m3

_CACHE = {}


def _build():
    nc = bacc.Bacc("TRN2", target_bir_lowering=False, debug=False, num_devices=8)

    def din(name, shape, dt=BF):
        return nc.declare_dram_parameter(name, list(shape), dt, isOutput=False)

    xnat = din("xnat", (128, NTILES * FT))        # x[b] [n,(f,t)] tiled: part=n%128
    xnatt = din("xnatt", (128, NTILES * FT))      # x[b] [n,(t,f)] tiled
    xftn = din("xftn", (FT, N))                   # [(f,t), n]
    xtfn = din("xtfn", (FT, N))                   # [(t,f), n]
    u1t = din("u1t", (128, NTILES))
    u2t = din("u2t", (128, NTILES * F))
    u3b = din("u3b", (128, 2), F32)
    w1 = din("w1", (T, 1))
    w2 = din("w2", (F, T))
    w3b = din("w3b", (128, 2), F32)
    bes = din("bes", (T, T), F32)
    vet = din("vet", (T, T), F32)
    ident = din("ident", (T, T), F32)
    bsb = din("bsb", (N, N), F8)                  # fp8 spatial-attn bias
    idDR = din("idDR", (128, 384), F8)            # [I|0|I] for DoubleRow bias-add
    idbf = din("idbf", (128, 128))                # identity bf16 (diag mask)
    vst = din("vst", (NTILES, 128, N), F8)        # per m-tile: SV*Vs^T fp8
    rowb = din("rowb", (128, NTILES), F32)        # 0.5*rowsum(Vs) per m
    chebb = din("chebb", (K - 1, N, N))           # cheb k=1,2 only (k=0 == I)
    thg = din("thg", (96, T * O))                 # zero-padded gcn lhsT per t
    twa = din("twa", (128, O))                    # [dt0|dt1] stacked tconv lhsT
    twb = din("twb", (88, T * O))                 # [dt2 ; res] per-t tconv lhsT
    biaso = din("biaso", (O, 1), F32)
    gfull = din("gfull", (128, OT))               # ln_g in (t,o) order, bf16
    bfull = din("bfull", (128, OT))               # ln_b in (t,o) order, bf16
    onesb = din("onesb", (128, 1))

    y_d = nc.declare_dram_parameter("y", [N, OT], BF, isOutput=True)  # (t,o) order
    # per-512-column scratch chunks [(t,o), 512] for exact LN pipelining deps
    ypd_c = [nc.dram_tensor("ypd%d" % c, [OT, 512], BF) for c in range(4)]

    with tile.TileContext(nc) as tc:
        from contextlib import ExitStack
        es = ExitStack()
        sm = es.enter_context(tc.tile_pool(name="sm", bufs=1))

        def load(dram, shape, dt=BF):
            t_ = sm.tile(list(shape), dt, tag=dram.name + "_s")
            nc.sync.dma_start(t_[:], dram[:])
            return t_

        xnat_s = load(xnat, (128, NTILES * FT))
        xnatt_s = load(xnatt, (128, NTILES * FT))

        xtfn_s = load(xtfn, (FT, N))
        u1t_s = load(u1t, (128, NTILES))
        u2t_s = load(u2t, (128, NTILES * F))
        u3b_s = load(u3b, (128, 2), F32)
        w1_s = load(w1, (T, 1))
        w2_s = load(w2, (F, T))
        w3b_s = load(w3b, (128, 2), F32)
        bes_s = load(bes, (T, T), F32)
        vet_s = load(vet, (T, T), F32)
        id_s = load(ident, (T, T), F32)
        idDR_s = load(idDR, (128, 384), F8)
        idb_s = load(idbf, (128, 128))
        rowb_s = load(rowb, (128, NTILES), F32)
        thg_s = load(thg, (96, T * O))
        twa_s = load(twa, (128, O))
        twb_s = load(twb, (88, T * O))
        biaso_s = load(biaso, (O, 1), F32)
        ones_s = load(onesb, (128, 1))

        # ================= temporal attention =================
        rhss_s = sm.tile([T, N], BF, tag="rhss")
        l2t_s = sm.tile([T, N], BF, tag="l2t")
        with tc.tile_pool(name="pst", bufs=1, space="PSUM") as pst, \
             tc.tile_pool(name="pat", bufs=1, space="PSUM") as pat, \
             tc.tile_pool(name="attp", bufs=1) as attp:
            xftn_f = []
            for f in range(F):
                t_ = attp.tile([T, N], BF, tag="xftn%d" % f)
                nc.sync.dma_start(t_[:], xftn[f * T:(f + 1) * T, :])
                xftn_f.append(t_)
            # xU1 [1, 24] = sum_n U1[n] * x_nat[n, (f,t)]
            xu1_ps = pst.tile([1, FT], F32, tag="tiny_ps")
            for j in range(NTILES):
                nc.tensor.matmul(xu1_ps[:], u1t_s[:, j:j + 1],
                                 xnat_s[:, j * FT:(j + 1) * FT],
                                 start=(j == 0), stop=(j == NTILES - 1))
            xu1_ft = sm.tile([F, T], F32, tag="xu1ft")
            xu1_row = attp.tile([1, FT], F32, tag="xu1row")
            nc.vector.tensor_copy(xu1_row[:], xu1_ps[:])
            for f in range(F):
                nc.sync.dma_start(xu1_ft[f:f + 1, :],
                                  xu1_row[0:1, f * T:(f + 1) * T])

            # rhs_t [n, u] (packed [128, (j,u)]) = x[.,0,u]*U3[0] + x[.,1,u]*U3[1]
            rhs_t = attp.tile([128, NTILES * T], BF, tag="rhs_t")
            xf0 = xnat_s[:].rearrange("p (j f t) -> p j f t", j=NTILES, f=F, t=T)
            tmp_rt = attp.tile([128, NTILES * T], BF, tag="tmp_rt")
            rt_v = tmp_rt[:].rearrange("p (j t) -> p j t", j=NTILES, t=T)
            rhs_tv = rhs_t[:].rearrange("p (j t) -> p j t", j=NTILES, t=T)
            nc.vector.tensor_scalar_mul(rt_v, xf0[:, :, 1, :], u3b_s[:, 1:2])
            nc.vector.scalar_tensor_tensor(rhs_tv, xf0[:, :, 0, :], u3b_s[:, 0:1],
                                           rt_v, op0=OP.mult, op1=OP.add)

            # M [f, u] = sum_n U2^T-tiles @ rhs_t
            m_ps = pst.tile([F, T], F32, tag="tiny_ps")
            for j in range(NTILES):
                nc.tensor.matmul(m_ps[:], u2t_s[:, j * F:(j + 1) * F],
                                 rhs_t[:, j * T:(j + 1) * T],
                                 start=(j == 0), stop=(j == NTILES - 1))
            m_s = sm.tile([F, T], F32, tag="m_s")
            nc.vector.tensor_copy(m_s[:], m_ps[:])

            # prod_t [t, u] = xu1_ft^T @ M ; PT = sigmoid(prod_t + be)
            pt_ps = pst.tile([T, T], F32, tag="tiny_ps")
            nc.tensor.matmul(pt_ps[:], xu1_ft[:], m_s[:], start=True, stop=True)
            pt_sb = sm.tile([T, T], F32, tag="pt_sb")
            nc.vector.tensor_add(pt_sb[:], pt_ps[:], bes_s[:])
            nc.scalar.activation(pt_sb[:], pt_sb[:], FN.Sigmoid)

            # E0 [t, v] = Ve @ PT  (lhsT = Ve^T)
            e0_ps = pst.tile([T, T], F32, tag="tiny_ps")
            nc.tensor.matmul(e0_ps[:], vet_s[:], pt_sb[:], start=True, stop=True)
            e0_s = sm.tile([T, T], F32, tag="e0_s")
            nc.vector.tensor_copy(e0_s[:], e0_ps[:])
            e0t_ps = pst.tile([T, T], F32, tag="tiny_ps")
            nc.tensor.transpose(e0t_ps[:], e0_s[:], id_s[:])
            e0t_s = sm.tile([T, T], F32, tag="e0t_s")
            nc.vector.tensor_copy(e0t_s[:], e0t_ps[:])
            # softmax along free (t) for each v
            mx = sm.tile([T, 1], F32, tag="mx")
            nc.vector.tensor_reduce(mx[:], e0t_s[:], axis=AX.X, op=OP.max)
            nmx = sm.tile([T, 1], F32, tag="nmx")
            nc.vector.tensor_scalar_mul(nmx[:], mx[:], -1.0)
            esum = sm.tile([T, 1], F32, tag="esum")
            nc.scalar.activation(e0t_s[:], e0t_s[:], FN.Exp, bias=nmx[:],
                                 scale=1.0, accum_out=esum[:])
            recs = sm.tile([T, 1], F32, tag="recs")
            nc.vector.reciprocal(recs[:], esum[:])
            nc.vector.tensor_scalar_mul(e0t_s[:], e0t_s[:], recs[:])
            esm_ps = pst.tile([T, T], F32, tag="tiny_ps")
            nc.tensor.transpose(esm_ps[:], e0t_s[:], id_s[:])
            esm_bf = sm.tile([T, T], BF, tag="esm_bf")
            nc.vector.tensor_copy(esm_bf[:], esm_ps[:])

            # xTA_ftn [(f,t), n] : per f: Esm^T @ x_ftn[f]
            xta_f = []
            for f in range(F):
                xta_t = attp.tile([T, N], BF, tag="xta%d" % f)
                xta_ps = pat.tile([T, N], F32, tag="attnps")
                for c in range(4):
                    nc.tensor.matmul(xta_ps[:, c * 512:(c + 1) * 512], esm_bf[:],
                                     xftn_f[f][:, c * 512:(c + 1) * 512],
                                     start=True, stop=True)
                nc.vector.tensor_copy(xta_t[:], xta_ps[:])
                xta_f.append(xta_t)

            # ============== spatial attention (small parts) ==============
            tmp_rs = attp.tile([T, N], BF, tag="tmp_rs")
            nc.vector.tensor_scalar_mul(tmp_rs[:], xta_f[1][:],
                                        w3b_s[0:T, 1:2])
            nc.vector.scalar_tensor_tensor(rhss_s[:], xta_f[0][:],
                                           w3b_s[0:T, 0:1], tmp_rs[:],
                                           op0=OP.mult, op1=OP.add)
            lsfn_s = sm.tile([F, N], BF, tag="lsfn")
            for f in range(F):
                ls_ps = pat.tile([1, N], F32, tag="attnps")
                for c in range(4):
                    nc.tensor.matmul(ls_ps[:, c * 512:(c + 1) * 512], w1_s[:],
                                     xta_f[f][:, c * 512:(c + 1) * 512],
                                     start=True, stop=True)
                if f == 0:
                    nc.vector.tensor_copy(lsfn_s[0:1, :], ls_ps[:])
                else:
                    ls1 = attp.tile([1, N], BF, tag="ls1")
                    nc.vector.tensor_copy(ls1[:], ls_ps[:])
                    nc.sync.dma_start(lsfn_s[1:2, :], ls1[:])
            l2_ps = pat.tile([T, N], F32, tag="attnps")
            for c in range(4):
                nc.tensor.matmul(l2_ps[:, c * 512:(c + 1) * 512], w2_s[:],
                                 lsfn_s[:, c * 512:(c + 1) * 512],
                                 start=True, stop=True)
            nc.vector.tensor_copy(l2t_s[:], l2_ps[:])

        # ========== Pc = tanh(0.5*(prod_s + bs)) in fp8 ==========
        from contextlib import ExitStack as _ES
        es2 = _ES()
        midp = es2.enter_context(tc.tile_pool(name="midp", bufs=1))
        rec_s = midp.tile([1, N], BF, tag="rec_s")
        rec_b = midp.tile([FT, N], BF, tag="rec_b")
        z_sb = midp.tile([96, N], BF, tag="z_sb")
        NPRE = 8  # cheb tiles prefetched into SBUF during the P phase
        with tc.tile_pool(name="Epool", bufs=1) as epool:
            E_s = epool.tile([128, NTILES * N], BF, tag="E")
            chpre = [epool.tile([128, N], BF, tag="chpre%d" % i,
                                name="chpre%d" % i)
                     for i in range(NPRE)]
            with tc.tile_pool(name="Ppool", bufs=1) as ppool:
                Pc_s = ppool.tile([128, NTILES * N], F8, tag="Pc")
                with tc.tile_pool(name="bsstr", bufs=3) as bsstr, \
                     tc.tile_pool(name="pps", bufs=2, space="PSUM") as pps:
                    for nt in range(NTILES):
                        bst = bsstr.tile([128, N], F8, tag="bst")
                        nc.sync.dma_start(bst[:], bsb[nt * 128:(nt + 1) * 128, :])
                        if nt < NPRE:  # prefetch cheb k=1 while DMA is idle
                            nc.sync.dma_start(
                                chpre[nt][:], chebb[0, nt * 128:(nt + 1) * 128, :])
                        pr_ps = pps.tile([128, N], F32, tag="pr_ps")
                        for c in range(4):
                            nc.tensor.matmul(pr_ps[:, c * 512:(c + 1) * 512],
                                             l2t_s[:, nt * 128:(nt + 1) * 128],
                                             rhss_s[:, c * 512:(c + 1) * 512],
                                             start=True, stop=False)
                            # DoubleRow identity-add of bs: lhsT [I|0]/[0|I]
                            base = (c // 2) * 1024
                            idv = idDR_s[:, (c % 2) * 128:(c % 2) * 128 + 256]
                            nc.tensor.matmul(
                                pr_ps[:, c * 512:(c + 1) * 512],
                                idv.rearrange("p (j m) -> p j m", j=2, m=128),
                                bst[:, base:base + 1024].rearrange(
                                    "p (j n) -> p j n", j=2, n=512),
                                start=False, stop=True, perf_mode=DR)
                        nc.scalar.activation(Pc_s[:, nt * N:(nt + 1) * N],
                                             pr_ps[:], FN.Tanh, scale=0.5)

                # == S_pre = SV*Vs @ Pc (fp8 DoubleRow); E = exp(psum/(2SV)+rowb)
                # colsum via DVE accumulate + one GPSIMD partition all-reduce.
                pc_v = Pc_s[:].rearrange("p (j n) -> p j n", j=NTILES, n=N)
                with tc.tile_pool(name="vstr", bufs=4) as vstr, \
                     tc.tile_pool(name="accp", bufs=1) as accp, \
                     tc.tile_pool(name="sps", bufs=4, space="PSUM") as sps:
                    acc_s = accp.tile([128, N], F32, tag="acc")
                    cs_s = accp.tile([128, N], F32, tag="cs")
                    for mt in range(NTILES):
                        vt = vstr.tile([128, N], F8, tag="vt")
                        nc.sync.dma_start(vt[:], vst[mt, :, :])
                        vt_v = vt[:].rearrange("p (j m) -> p j m", j=NTILES, m=128)
                        for h in range(2):
                            s_ps = sps.tile([128, N // 2], F32, tag="s_ps")
                            for c in range(2):
                                col = h * 1024 + c * 512
                                for jp in range(NTILES // 2):
                                    nc.tensor.matmul(
                                        s_ps[:, c * 512:(c + 1) * 512],
                                        vt_v[:, 2 * jp:2 * jp + 2, :],
                                        pc_v[:, 2 * jp:2 * jp + 2,
                                             col:col + 512],
                                        start=(jp == 0),
                                        stop=(jp == NTILES // 2 - 1),
                                        perf_mode=DR)
                            ev = E_s[:, mt * N + h * 1024:
                                     mt * N + (h + 1) * 1024]
                            nc.scalar.activation(ev, s_ps[:], FN.Exp,
                                                 bias=rowb_s[:, mt:mt + 1],
                                                 scale=1.0 / (2.0 * SV))
                        if mt == 0:
                            nc.vector.tensor_copy(
                                acc_s[:], E_s[:, 0:N])
                        else:
                            nc.vector.tensor_add(
                                acc_s[:], acc_s[:], E_s[:, mt * N:(mt + 1) * N])
                    import concourse.bass_isa as bass_isa
                    nc.gpsimd.partition_all_reduce(
                        cs_s[:], acc_s[:], channels=128,
                        reduce_op=bass_isa.ReduceOp.add)
                    with nc.allow_low_precision(reason="colsum ~2048*E; bf16 "
                                                "rec only scales softmax"):
                        nc.vector.reciprocal(rec_s[:], cs_s[0:1, :])
                nc.gpsimd.partition_broadcast(rec_b[:], rec_s[:])

            # ========== cheb: Z_k = x_natt^T @ (cheb_k * E) ==========
            # k=0: cheb_0 == I, so only diagonal blocks of E contribute.
            nc.vector.memset(z_sb[:], 0.0)
            with tc.tile_pool(name="chstr", bufs=12) as chstr, \
                 tc.tile_pool(name="wtmp", bufs=6) as wtmp, \
                 tc.tile_pool(name="zps", bufs=2, space="PSUM") as zps:
                z0_ps = zps.tile([FT, N], F32, tag="z_ps")
                for mt in range(NTILES):
                    wt0 = wtmp.tile([128, 128], BF, tag="wt0")
                    d0 = mt * N + mt * 128
                    nc.vector.tensor_mul(wt0[:], E_s[:, d0:d0 + 128], idb_s[:])
                    nc.tensor.matmul(z0_ps[:, mt * 128:(mt + 1) * 128],
                                     xnatt_s[:, mt * FT:(mt + 1) * FT],
                                     wt0[:], start=True, stop=True)
                nc.vector.tensor_mul(z_sb[0:FT, :], z0_ps[:], rec_b[:])
                for k in range(1, K):
                    z_ps = zps.tile([FT, N], F32, tag="z_ps")
                    for mt in range(NTILES):
                        if k == 1 and mt < NPRE:
                            cht_ap = chpre[mt][:]
                        else:
                            cht = chstr.tile([128, N], BF, tag="cht")
                            nc.sync.dma_start(
                                cht[:], chebb[k - 1, mt * 128:(mt + 1) * 128, :])
                            cht_ap = cht[:]
                        wt = wtmp.tile([128, N], BF, tag="wt")
                        eng = nc.gpsimd if (mt % 4) == 3 else nc.vector
                        eng.tensor_mul(wt[:], cht_ap,
                                       E_s[:, mt * N:(mt + 1) * N])
                        for c in range(4):
                            nc.tensor.matmul(
                                z_ps[:, c * 512:(c + 1) * 512],
                                xnatt_s[:, mt * FT:(mt + 1) * FT],
                                wt[:, c * 512:(c + 1) * 512],
                                start=(mt == 0), stop=(mt == NTILES - 1))
                    nc.vector.tensor_mul(z_sb[32 * k:32 * k + FT, :], z_ps[:],
                                         rec_b[:])

        # ================= gcn =================
        with tc.tile_pool(name="gcnp", bufs=1) as gcnp, \
             tc.tile_pool(name="ypp", bufs=2) as ypp, \
             tc.tile_pool(name="ynp", bufs=4) as ynp, \
             tc.tile_pool(name="lns", bufs=4) as lns:
            # gcnB rows 0-63: gcn_pad blocks 0..13 ([0,g0..g11,0]); rows
            # 64-127: gcn_pad blocks shifted by one ([g0..g11,0,-]).
            gcnB = gcnp.tile([128, (T + 2) * N], BF, tag="gcnB")
            # gcnC rows 0-63: gcn_pad blocks shifted by two; rows 64-87: xtfn
            # replicated at every t block (residual-conv input).
            gcnC = gcnp.tile([88, T * N], BF, tag="gcnC")
            nc.vector.memset(gcnB[0:64, 0:N], 0.0)
            nc.vector.memset(gcnB[0:64, (T + 1) * N:(T + 2) * N], 0.0)
            for t in range(T):
                nc.sync.dma_start(gcnC[64:88, t * N:(t + 1) * N], xtfn_s[:])
            with tc.tile_pool(name="gps", bufs=2, space="PSUM") as gps:
                for t in range(T):
                    g_ps = gps.tile([O, N], F32, tag="g_ps")
                    for c in range(4):
                        nc.tensor.matmul(
                            g_ps[:, c * 512:(c + 1) * 512],
                            thg_s[:, t * O:(t + 1) * O],
                            z_sb[:, c * 512:(c + 1) * 512],
                            start=True, stop=True)
                    if t % 2 == 0:
                        nc.scalar.activation(
                            gcnB[0:64, (t + 1) * N:(t + 2) * N],
                            g_ps[:], FN.Relu)
                    else:
                        nc.vector.tensor_scalar_max(
                            gcnB[0:64, (t + 1) * N:(t + 2) * N],
                            g_ps[:], 0.0)
                    # shifted copies for the packed-contraction tconv
                    nc.sync.dma_start(gcnB[64:128, t * N:(t + 1) * N],
                                      gcnB[0:64, (t + 1) * N:(t + 2) * N])
                    if t >= 2:
                        nc.sync.dma_start(
                            gcnC[0:64, (t - 2) * N:(t - 1) * N],
                            gcnB[0:64, t * N:(t + 1) * N])
            for t in (T - 2, T - 1):
                nc.sync.dma_start(gcnC[0:64, t * N:(t + 1) * N],
                                  gcnB[0:64, (t + 2) * N:(t + 3) * N])

            # ====== tconv + res, with layernorm interleaved per column-chunk
            gfull_s = gcnp.tile([128, OT], BF, tag="gfull_s")
            nc.sync.dma_start(gfull_s[:], gfull[:])
            bfull_s = gcnp.tile([128, OT], BF, tag="bfull_s")
            nc.sync.dma_start(bfull_s[:], bfull[:])
            epsb = gcnp.tile([128, 1], F32, tag="epsb")
            nc.vector.memset(epsb[:], float(O) * LN_EPS)
            with tc.tile_pool(name="tps", bufs=8, space="PSUM") as tps:
                for c in range(4):
                    ypre_s = ypp.tile([O, T * 512], BF, tag="ypre")
                    for t in range(T):
                        tc_ps = tps.tile([O, 512], F32, tag="tc_ps")
                        nc.tensor.matmul(
                            tc_ps[:], twa_s[:],
                            gcnB[:, t * N + c * 512:t * N + (c + 1) * 512],
                            start=True, stop=False)
                        nc.tensor.matmul(
                            tc_ps[:], twb_s[:, t * O:(t + 1) * O],
                            gcnC[:, t * N + c * 512:t * N + (c + 1) * 512],
                            start=False, stop=True)
                        if t % 2 == 0:
                            nc.scalar.activation(
                                ypre_s[:, t * 512:(t + 1) * 512],
                                tc_ps[:], FN.Relu, bias=biaso_s[:], scale=1.0)
                        else:
                            nc.vector.tensor_scalar(
                                ypre_s[:, t * 512:(t + 1) * 512], tc_ps[:],
                                biaso_s[:], 0.0, op0=OP.add, op1=OP.max)
                    ypdv = ypd_c[c][:].rearrange("(t o) n -> o t n", o=O, t=T)
                    nc.sync.dma_start(
                        ypdv[:, :, :],
                        ypre_s[:].rearrange("o (t n) -> o t n", t=T))
                    # ---- layernorm for the 4 n-tiles of this chunk ----
                    for q in range(4):
                        nt = c * 4 + q
                        yt = ynp.tile([128, OT], BF, tag="yt")
                        nc.sync.dma_start_transpose(
                            yt[:], ypd_c[c][:, q * 128:(q + 1) * 128])
                        ytv = yt[:].rearrange("p (t o) -> p t o", o=O, t=T)
                        mus = lns.tile([128, T], F32, tag="mus")
                        nc.vector.tensor_reduce(mus[:], ytv, axis=AX.X,
                                                op=OP.add)
                        mud = lns.tile([128, T], F32, tag="mud")
                        nc.gpsimd.tensor_scalar_mul(mud[:], mus[:], 1.0 / O)
                        sqf = ynp.tile([128, OT], F32, tag="sqf")
                        nc.scalar.activation(sqf[:], yt[:], FN.Square)
                        sqs = lns.tile([128, T], F32, tag="sqs")
                        nc.vector.tensor_reduce(
                            sqs[:], sqf[:].rearrange("p (t o) -> p t o",
                                                     o=O, t=T),
                            axis=AX.X, op=OP.add)
                        w_t = lns.tile([128, T], F32, tag="w_t")
                        nc.gpsimd.tensor_mul(w_t[:], mus[:], mud[:])
                        nc.gpsimd.tensor_sub(w_t[:], sqs[:], w_t[:])  # 64*var
                        s_t = lns.tile([128, T], F32, tag="s_t")
                        nc.scalar.activation(s_t[:], w_t[:], FN.Sqrt,
                                             bias=epsb[:])
                        r_t = lns.tile([128, T], F32, tag="r_t")
                        nc.vector.reciprocal(r_t[:], s_t[:])  # isig/8
                        nmr = lns.tile([128, T], F32, tag="nmr")
                        nc.gpsimd.tensor_scalar_mul(nmr[:], mud[:], -1.0)
                        nc.gpsimd.tensor_mul(nmr[:], nmr[:], r_t[:])
                        yh = ynp.tile([128, OT], BF, tag="yh")
                        yhv = yh[:].rearrange("p (t o) -> p t o", o=O, t=T)
                        for t in range(T):
                            if t % 2 == 1:
                                nc.scalar.activation(
                                    yhv[:, t, :], ytv[:, t, :], FN.Identity,
                                    bias=nmr[:, t:t + 1], scale=r_t[:, t:t + 1])
                            else:
                                nc.vector.tensor_scalar(
                                    yhv[:, t, :], ytv[:, t, :],
                                    mud[:, t:t + 1], r_t[:, t:t + 1],
                                    op0=OP.subtract, op1=OP.mult)
                        yg = ynp.tile([128, OT], BF, tag="yg")
                        nc.vector.tensor_mul(yg[:], yh[:], gfull_s[:])
                        nc.gpsimd.tensor_add(yg[:], yg[:], bfull_s[:])
                        nc.sync.dma_start(y_d[nt * 128:(nt + 1) * 128, :],
                                          yg[:])
        es2.close()

        es.close()
    nc.compile()
    return nc


def _pack(x_b, consts):
    m = dict(consts)
    xb = np.asarray(x_b, np.float32)
    x_nat = xb.reshape(N, FT)
    x_natt = np.ascontiguousarray(xb.transpose(0, 2, 1)).reshape(N, FT)
    pk = lambda a: np.ascontiguousarray(
        a.reshape(NTILES, 128, FT).transpose(1, 0, 2).reshape(128, NTILES * FT)
    ).astype(bf16)
    m["xnat"] = pk(x_nat)
    m["xnatt"] = pk(x_natt)
    m["xftn"] = np.ascontiguousarray(xb.transpose(1, 2, 0).reshape(FT, N)).astype(bf16)
    m["xtfn"] = np.ascontiguousarray(xb.transpose(2, 1, 0).reshape(FT, N)).astype(bf16)
    return m


def kernel(**inputs):
    if "nc" not in _CACHE:
        _CACHE["nc"] = _build()
    nc = _CACHE["nc"]

    f32 = lambda a: np.asarray(a, np.float32)
    x = f32(inputs["x"])
    U1, U2, U3 = f32(inputs["U1"]), f32(inputs["U2"]), f32(inputs["U3"])
    Vs, bs = f32(inputs["Vs"]), f32(inputs["bs"])
    cheb = f32(inputs["cheb"])
    Theta = f32(inputs["Theta"])
    W1, W2, W3 = f32(inputs["W1"]), f32(inputs["W2"]), f32(inputs["W3"])
    be, Ve = f32(inputs["be"]), f32(inputs["Ve"])
    tw, tb = f32(inputs["tconv_w"]), f32(inputs["tconv_b"])
    rw, rb = f32(inputs["rconv_w"]), f32(inputs["rconv_b"])
    g, bb = f32(inputs["ln_g"]), f32(inputs["ln_b"])

    consts = {}
    consts["u1t"] = np.ascontiguousarray(U1.reshape(NTILES, 128).T).astype(bf16)
    consts["u2t"] = np.ascontiguousarray(
        U2.T.reshape(NTILES, 128, F).transpose(1, 0, 2).reshape(128, NTILES * F)
    ).astype(bf16)
    consts["u3b"] = np.broadcast_to(U3[None, :], (128, 2)).copy()
    consts["w1"] = W1[:, None].astype(bf16)
    consts["w2"] = W2.astype(bf16)
    consts["w3b"] = np.broadcast_to(W3[None, :], (128, 2)).copy()
    consts["bes"] = np.ascontiguousarray(be[0])
    consts["vet"] = np.ascontiguousarray(Ve.T)
    consts["ident"] = np.eye(T, dtype=np.float32)
    idDR = np.zeros((128, 384), np.float32)
    idDR[:, 0:128] = np.eye(128)
    idDR[:, 256:384] = np.eye(128)
    consts["idDR"] = idDR.astype(f8e4)
    consts["idbf"] = np.eye(128, dtype=np.float32).astype(bf16)
    consts["bsb"] = bs[0].astype(f8e4)
    VsT = np.ascontiguousarray(Vs.T)  # [n, m]
    consts["vst"] = np.ascontiguousarray(
        (SV * VsT).reshape(NTILES, 128, NTILES, 128).transpose(2, 1, 0, 3)
        .reshape(NTILES, 128, N)
    ).astype(f8e4)
    consts["rowb"] = np.ascontiguousarray(
        (0.5 * Vs.sum(axis=1)).reshape(NTILES, 128).T).astype(np.float32)
    consts["chebb"] = cheb[1:].astype(bf16)
    thg = np.zeros((96, T * O), np.float32)
    for t in range(T):
        for k in range(K):
            for f in range(F):
                thg[32 * k + t * F + f, t * O:(t + 1) * O] = Theta[k, f]
    consts["thg"] = thg.astype(bf16)
    # twa: [tw_dt0 ; tw_dt1] stacked along contraction; twb: per-t
    # [tw_dt2 (64 rows) ; residual-conv rows (24)]
    twl = tw[:, :, 0, :].transpose(1, 2, 0).reshape(O, 3 * O)  # [i, (dt, o)]
    consts["twa"] = np.concatenate(
        [twl[:, 0:O], twl[:, O:2 * O]], axis=0).astype(bf16)
    twb = np.zeros((88, T * O), np.float32)
    for t in range(T):
        twb[0:O, t * O:(t + 1) * O] = twl[:, 2 * O:3 * O]
        for f in range(F):
            twb[O + t * F + f, t * O:(t + 1) * O] = rw[:, f, 0, 0]
    consts["twb"] = twb.astype(bf16)

    consts["biaso"] = (tb + rb)[:, None]
    consts["gfull"] = np.broadcast_to(
        (8.0 * np.tile(g, T))[None, :], (128, OT)).astype(bf16).copy()
    consts["bfull"] = np.broadcast_to(
        np.tile(bb, T)[None, :], (128, OT)).astype(bf16).copy()
    consts["onesb"] = np.ones((128, 1), bf16)

    in_maps = [_pack(x[b], consts) for b in range(B)]
    _CACHE["im"] = in_maps
    res = run_bass_kernel_spmd(nc, in_maps, list(range(B)))
    out = np.stack([
        np.asarray(res.results[b]["y"], np.float32)
        .reshape(N, T, O).transpose(0, 2, 1)
        for b in range(B)
    ])
    return out.astype(np.float32)


# revision 70
# speedup vs baseline: 1.1538x; 1.0424x over previous
import sys
if '/opt/trn_rl_repo' not in sys.path:
    sys.path.insert(0, '/opt/trn_rl_repo')
import numpy as np
import ml_dtypes

import concourse.bass as bass
import concourse.bacc as bacc
import concourse.tile as tile
from concourse import mybir
from concourse.bass_utils import run_bass_kernel_spmd

BF = mybir.dt.bfloat16
F32 = mybir.dt.float32
F8 = mybir.dt.float8e4
FN = mybir.ActivationFunctionType
OP = mybir.AluOpType
AX = mybir.AxisListType
DR = mybir.MatmulPerfMode.DoubleRow

B, N, F, T = 8, 2048, 2, 12
K, O = 3, 64
FT = F * T           # 24
NTILES = N // 128    # 16
OT = O * T           # 768
LN_EPS = 1e-5
SV = 8.0             # scale folded into fp8 Vs (S_pre = psum/(2*SV) + rowb)

bf16 = ml_dtypes.bfloat16
f8e4 = ml_dtypes.float8_e4m3

_CACHE = {}


def _build():
    nc = bacc.Bacc("TRN2", target_bir_lowering=False, debug=False, num_devices=8)

    def din(name, shape, dt=BF):
        return nc.declare_dram_parameter(name, list(shape), dt, isOutput=False)

    xnat = din("xnat", (128, NTILES * FT))        # x[b] [n,(f,t)] tiled: part=n%128
    xnatt = din("xnatt", (128, NTILES * FT))      # x[b] [n,(t,f)] tiled
    xftn = din("xftn", (FT, N))                   # [(f,t), n]
    xtfn = din("xtfn", (FT, N))                   # [(t,f), n]
    u1t = din("u1t", (128, NTILES))
    u2t = din("u2t", (128, NTILES * F))
    u3b = din("u3b", (128, 2), F32)
    w1 = din("w1", (T, 1))
    w2 = din("w2", (F, T))
    w3b = din("w3b", (128, 2), F32)
    bes = din("bes", (T, T), F32)
    vet = din("vet", (T, T), F32)
    ident = din("ident", (T, T), F32)
    bsb = din("bsb", (N, N), F8)                  # fp8 spatial-attn bias
    idDR = din("idDR", (128, 384), F8)            # [I|0|I] for DoubleRow bias-add
    idbf = din("idbf", (128, 128))                # identity bf16 (diag mask)
    vst = din("vst", (NTILES, 128, N), F8)        # per m-tile: SV*Vs^T fp8
    rowb = din("rowb", (128, NTILES), F32)        # 0.5*rowsum(Vs) per m
    chebb = din("chebb", (K - 1, N, N))           # cheb k=1,2 only (k=0 == I)
    thg = din("thg", (96, T * O))                 # zero-padded gcn lhsT per t
    twa = din("twa", (128, O))                    # [dt0|dt1] stacked tconv lhsT
    tw2p = din("tw2p", (128, O))                  # tw_dt2 at partitions 64:128
    rwp = din("rwp", (FT, T * O))                 # zero-padded res lhsT per t
    biaso = din("biaso", (O, 1), F32)
    gfull = din("gfull", (128, OT))               # ln_g in (t,o) order, bf16
    bfull = din("bfull", (128, OT))               # ln_b in (t,o) order, bf16
    onesb = din("onesb", (128, 1))

    y_d = nc.declare_dram_parameter("y", [N, OT], BF, isOutput=True)  # (t,o) order
    # per-512-column scratch chunks [(t,o), 512] for exact LN pipelining deps
    ypd_c = [nc.dram_tensor("ypd%d" % c, [OT, 512], BF) for c in range(4)]

    with tile.TileContext(nc) as tc:
        from contextlib import ExitStack
        es = ExitStack()
        sm = es.enter_context(tc.tile_pool(name="sm", bufs=1))

        def load(dram, shape, dt=BF):
            t_ = sm.tile(list(shape), dt, tag=dram.name + "_s")
            nc.sync.dma_start(t_[:], dram[:])
            return t_

        xnat_s = load(xnat, (128, NTILES * FT))
        xnatt_s = load(xnatt, (128, NTILES * FT))

        xtfn_s = load(xtfn, (FT, N))
        u1t_s = load(u1t, (128, NTILES))
        u2t_s = load(u2t, (128, NTILES * F))
        u3b_s = load(u3b, (128, 2), F32)
        w1_s = load(w1, (T, 1))
        w2_s = load(w2, (F, T))
        w3b_s = load(w3b, (128, 2), F32)
        bes_s = load(bes, (T, T), F32)
        vet_s = load(vet, (T, T), F32)
        id_s = load(ident, (T, T), F32)
        idDR_s = load(idDR, (128, 384), F8)
        idb_s = load(idbf, (128, 128))
        rowb_s = load(rowb, (128, NTILES), F32)
        thg_s = load(thg, (96, T * O))
        twa_s = load(twa, (128, O))
        tw2p_s = load(tw2p, (128, O))
        rwp_s = load(rwp, (FT, T * O))
        biaso_s = load(biaso, (O, 1), F32)
        ones_s = load(onesb, (128, 1))

        # ================= temporal attention =================
        rhss_s = sm.tile([T, N], BF, tag="rhss")
        l2t_s = sm.tile([T, N], BF, tag="l2t")
        with tc.tile_pool(name="pst", bufs=1, space="PSUM") as pst, \
             tc.tile_pool(name="pat", bufs=1, space="PSUM") as pat, \
             tc.tile_pool(name="attp", bufs=1) as attp:
            xftn_f = []
            for f in range(F):
                t_ = attp.tile([T, N], BF, tag="xftn%d" % f)
                nc.sync.dma_start(t_[:], xftn[f * T:(f + 1) * T, :])
                xftn_f.append(t_)
            # xU1 [1, 24] = sum_n U1[n] * x_nat[n, (f,t)]
            xu1_ps = pst.tile([1, FT], F32, tag="tiny_ps")
            for j in range(NTILES):
                nc.tensor.matmul(xu1_ps[:], u1t_s[:, j:j + 1],
                                 xnat_s[:, j * FT:(j + 1) * FT],
                                 start=(j == 0), stop=(j == NTILES - 1))
            xu1_ft = sm.tile([F, T], F32, tag="xu1ft")
            xu1_row = attp.tile([1, FT], F32, tag="xu1row")
            nc.vector.tensor_copy(xu1_row[:], xu1_ps[:])
            for f in range(F):
                nc.sync.dma_start(xu1_ft[f:f + 1, :],
                                  xu1_row[0:1, f * T:(f + 1) * T])

            # rhs_t [n, u] (packed [128, (j,u)]) = x[.,0,u]*U3[0] + x[.,1,u]*U3[1]
            rhs_t = attp.tile([128, NTILES * T], BF, tag="rhs_t")
            xf0 = xnat_s[:].rearrange("p (j f t) -> p j f t", j=NTILES, f=F, t=T)
            tmp_rt = attp.tile([128, NTILES * T], BF, tag="tmp_rt")
            rt_v = tmp_rt[:].rearrange("p (j t) -> p j t", j=NTILES, t=T)
            rhs_tv = rhs_t[:].rearrange("p (j t) -> p j t", j=NTILES, t=T)
            nc.vector.tensor_scalar_mul(rt_v, xf0[:, :, 1, :], u3b_s[:, 1:2])
            nc.vector.scalar_tensor_tensor(rhs_tv, xf0[:, :, 0, :], u3b_s[:, 0:1],
                                           rt_v, op0=OP.mult, op1=OP.add)

            # M [f, u] = sum_n U2^T-tiles @ rhs_t
            m_ps = pst.tile([F, T], F32, tag="tiny_ps")
            for j in range(NTILES):
                nc.tensor.matmul(m_ps[:], u2t_s[:, j * F:(j + 1) * F],
                                 rhs_t[:, j * T:(j + 1) * T],
                                 start=(j == 0), stop=(j == NTILES - 1))
            m_s = sm.tile([F, T], F32, tag="m_s")
            nc.vector.tensor_copy(m_s[:], m_ps[:])

            # prod_t [t, u] = xu1_ft^T @ M ; PT = sigmoid(prod_t + be)
            pt_ps = pst.tile([T, T], F32, tag="tiny_ps")
            nc.tensor.matmul(pt_ps[:], xu1_ft[:], m_s[:], start=True, stop=True)
            pt_sb = sm.tile([T, T], F32, tag="pt_sb")
            nc.vector.tensor_add(pt_sb[:], pt_ps[:], bes_s[:])
            nc.scalar.activation(pt_sb[:], pt_sb[:], FN.Sigmoid)

            # E0 [t, v] = Ve @ PT  (lhsT = Ve^T)
            e0_ps = pst.tile([T, T], F32, tag="tiny_ps")
            nc.tensor.matmul(e0_ps[:], vet_s[:], pt_sb[:], start=True, stop=True)
            e0_s = sm.tile([T, T], F32, tag="e0_s")
            nc.vector.tensor_copy(e0_s[:], e0_ps[:])
            e0t_ps = pst.tile([T, T], F32, tag="tiny_ps")
            nc.tensor.transpose(e0t_ps[:], e0_s[:], id_s[:])
            e0t_s = sm.tile([T, T], F32, tag="e0t_s")
            nc.vector.tensor_copy(e0t_s[:], e0t_ps[:])
            # softmax along free (t) for each v
            mx = sm.tile([T, 1], F32, tag="mx")
            nc.vector.tensor_reduce(mx[:], e0t_s[:], axis=AX.X, op=OP.max)
            nmx = sm.tile([T, 1], F32, tag="nmx")
            nc.vector.tensor_scalar_mul(nmx[:], mx[:], -1.0)
            esum = sm.tile([T, 1], F32, tag="esum")
            nc.scalar.activation(e0t_s[:], e0t_s[:], FN.Exp, bias=nmx[:],
                                 scale=1.0, accum_out=esum[:])
            recs = sm.tile([T, 1], F32, tag="recs")
            nc.vector.reciprocal(recs[:], esum[:])
            nc.vector.tensor_scalar_mul(e0t_s[:], e0t_s[:], recs[:])
            esm_ps = pst.tile([T, T], F32, tag="tiny_ps")
            nc.tensor.transpose(esm_ps[:], e0t_s[:], id_s[:])
            esm_bf = sm.tile([T, T], BF, tag="esm_bf")
            nc.vector.tensor_copy(esm_bf[:], esm_ps[:])

            # xTA_ftn [(f,t), n] : per f: Esm^T @ x_ftn[f]
            xta_f = []
            for f in range(F):
                xta_t = attp.tile([T, N], BF, tag="xta%d" % f)
                xta_ps = pat.tile([T, N], F32, tag="attnps")
                for c in range(4):
                    nc.tensor.matmul(xta_ps[:, c * 512:(c + 1) * 512], esm_bf[:],
                                     xftn_f[f][:, c * 512:(c + 1) * 512],
                                     start=True, stop=True)
                nc.vector.tensor_copy(xta_t[:], xta_ps[:])
                xta_f.append(xta_t)

            # ============== spatial attention (small parts) ==============
            tmp_rs = attp.tile([T, N], BF, tag="tmp_rs")
            nc.vector.tensor_scalar_mul(tmp_rs[:], xta_f[1][:],
                                        w3b_s[0:T, 1:2])
            nc.vector.scalar_tensor_tensor(rhss_s[:], xta_f[0][:],
                                           w3b_s[0:T, 0:1], tmp_rs[:],
                                           op0=OP.mult, op1=OP.add)
            lsfn_s = sm.tile([F, N], BF, tag="lsfn")
            for f in range(F):
                ls_ps = pat.tile([1, N], F32, tag="attnps")
                for c in range(4):
                    nc.tensor.matmul(ls_ps[:, c * 512:(c + 1) * 512], w1_s[:],
                                     xta_f[f][:, c * 512:(c + 1) * 512],
                                     start=True, stop=True)
                if f == 0:
                    nc.vector.tensor_copy(lsfn_s[0:1, :], ls_ps[:])
                else:
                    ls1 = attp.tile([1, N], BF, tag="ls1")
                    nc.vector.tensor_copy(ls1[:], ls_ps[:])
                    nc.sync.dma_start(lsfn_s[1:2, :], ls1[:])
            l2_ps = pat.tile([T, N], F32, tag="attnps")
            for c in range(4):
                nc.tensor.matmul(l2_ps[:, c * 512:(c + 1) * 512], w2_s[:],
                                 lsfn_s[:, c * 512:(c + 1) * 512],
                                 start=True, stop=True)
            nc.vector.tensor_copy(l2t_s[:], l2_ps[:])

        # ========== Pc = tanh(0.5*(prod_s + bs)) in fp8 ==========
        from contextlib import ExitStack as _ES
        es2 = _ES()
        midp = es2.enter_context(tc.tile_pool(name="midp", bufs=1))
        rec_s = midp.tile([1, N], BF, tag="rec_s")
        rec_b = midp.tile([FT, N], BF, tag="rec_b")
        z_sb = midp.tile([96, N], BF, tag="z_sb")
        NPRE = 8  # cheb tiles prefetched into SBUF during the P phase
        with tc.tile_pool(name="Epool", bufs=1) as epool:
            E_s = epool.tile([128, NTILES * N], BF, tag="E")
            chpre = [epool.tile([128, N], BF, tag="chpre%d" % i,
                                name="chpre%d" % i)
                     for i in range(NPRE)]
            with tc.tile_pool(name="Ppool", bufs=1) as ppool:
                Pc_s = ppool.tile([128, NTILES * N], F8, tag="Pc")
                with tc.tile_pool(name="bsstr", bufs=3) as bsstr, \
                     tc.tile_pool(name="pps", bufs=2, space="PSUM") as pps:
                    for nt in range(NTILES):
                        bst = bsstr.tile([128, N], F8, tag="bst")
                        nc.sync.dma_start(bst[:], bsb[nt * 128:(nt + 1) * 128, :])
                        if nt < NPRE:  # prefetch cheb k=1 while DMA is idle
                            nc.sync.dma_start(
                                chpre[nt][:], chebb[0, nt * 128:(nt + 1) * 128, :])
                        pr_ps = pps.tile([128, N], F32, tag="pr_ps")
                        for c in range(4):
                            nc.tensor.matmul(pr_ps[:, c * 512:(c + 1) * 512],
                                             l2t_s[:, nt * 128:(nt + 1) * 128],
                                             rhss_s[:, c * 512:(c + 1) * 512],
                                             start=True, stop=False)
                            # DoubleRow identity-add of bs: lhsT [I|0]/[0|I]
                            base = (c // 2) * 1024
                            idv = idDR_s[:, (c % 2) * 128:(c % 2) * 128 + 256]
                            nc.tensor.matmul(
                                pr_ps[:, c * 512:(c + 1) * 512],
                                idv.rearrange("p (j m) -> p j m", j=2, m=128),
                                bst[:, base:base + 1024].rearrange(
                                    "p (j n) -> p j n", j=2, n=512),
                                start=False, stop=True, perf_mode=DR)
                        nc.scalar.activation(Pc_s[:, nt * N:(nt + 1) * N],
                                             pr_ps[:], FN.Tanh, scale=0.5)

                # == S_pre = SV*Vs @ Pc (fp8 DoubleRow); E = exp(psum/(2SV)+rowb)
                # colsum via DVE accumulate + one GPSIMD partition all-reduce.
                pc_v = Pc_s[:].rearrange("p (j n) -> p j n", j=NTILES, n=N)
                with tc.tile_pool(name="vstr", bufs=4) as vstr, \
                     tc.tile_pool(name="accp", bufs=1) as accp, \
                     tc.tile_pool(name="sps", bufs=4, space="PSUM") as sps:
                    acc_s = accp.tile([128, N], F32, tag="acc")
                    cs_s = accp.tile([128, N], F32, tag="cs")
                    for mt in range(NTILES):
                        vt = vstr.tile([128, N], F8, tag="vt")
                        nc.sync.dma_start(vt[:], vst[mt, :, :])
                        vt_v = vt[:].rearrange("p (j m) -> p j m", j=NTILES, m=128)
                        for h in range(2):
                            s_ps = sps.tile([128, N // 2], F32, tag="s_ps")
                            for c in range(2):
                                col = h * 1024 + c * 512
                                for jp in range(NTILES // 2):
                                    nc.tensor.matmul(
                                        s_ps[:, c * 512:(c + 1) * 512],
                                        vt_v[:, 2 * jp:2 * jp + 2, :],
                                        pc_v[:, 2 * jp:2 * jp + 2,
                                             col:col + 512],
                                        start=(jp == 0),
                                        stop=(jp == NTILES // 2 - 1),
                                        perf_mode=DR)
                            ev = E_s[:, mt * N + h * 1024:
                                     mt * N + (h + 1) * 1024]
                            nc.scalar.activation(ev, s_ps[:], FN.Exp,
                                                 bias=rowb_s[:, mt:mt + 1],
                                                 scale=1.0 / (2.0 * SV))
                        if mt == 0:
                            nc.vector.tensor_copy(
                                acc_s[:], E_s[:, 0:N])
                        else:
                            nc.vector.tensor_add(
                                acc_s[:], acc_s[:], E_s[:, mt * N:(mt + 1) * N])
                    import concourse.bass_isa as bass_isa
                    nc.gpsimd.partition_all_reduce(
                        cs_s[:], acc_s[:], channels=128,
                        reduce_op=bass_isa.ReduceOp.add)
                    with nc.allow_low_precision(reason="colsum ~2048*E; bf16 "
                                                "rec only scales softmax"):
                        nc.vector.reciprocal(rec_s[:], cs_s[0:1, :])
                nc.gpsimd.partition_broadcast(rec_b[:], rec_s[:])

            # ========== cheb: Z_k = x_natt^T @ (cheb_k * E) ==========
            # k=0: cheb_0 == I, so only diagonal blocks of E contribute.
            nc.vector.memset(z_sb[:], 0.0)
            with tc.tile_pool(name="chstr", bufs=12) as chstr, \
                 tc.tile_pool(name="wtmp", bufs=6) as wtmp, \
                 tc.tile_pool(name="zps", bufs=2, space="PSUM") as zps:
                z0_ps = zps.tile([FT, N], F32, tag="z_ps")
                for mt in range(NTILES):
                    wt0 = wtmp.tile([128, 128], BF, tag="wt0")
                    d0 = mt * N + mt * 128
                    nc.vector.tensor_mul(wt0[:], E_s[:, d0:d0 + 128], idb_s[:])
                    nc.tensor.matmul(z0_ps[:, mt * 128:(mt + 1) * 128],
                                     xnatt_s[:, mt * FT:(mt + 1) * FT],
                                     wt0[:], start=True, stop=True)
                nc.vector.tensor_mul(z_sb[0:FT, :], z0_ps[:], rec_b[:])
                for k in range(1, K):
                    z_ps = zps.tile([FT, N], F32, tag="z_ps")
                    for mt in range(NTILES):
                        if k == 1 and mt < NPRE:
                            cht_ap = chpre[mt][:]
                        else:
                            cht = chstr.tile([128, N], BF, tag="cht")
                            nc.sync.dma_start(
                                cht[:], chebb[k - 1, mt * 128:(mt + 1) * 128, :])
                            cht_ap = cht[:]
                        wt = wtmp.tile([128, N], BF, tag="wt")
                        eng = nc.gpsimd if (mt % 4) == 3 else nc.vector
                        eng.tensor_mul(wt[:], cht_ap,
                                       E_s[:, mt * N:(mt + 1) * N])
                        for c in range(4):
                            nc.tensor.matmul(
                                z_ps[:, c * 512:(c + 1) * 512],
                                xnatt_s[:, mt * FT:(mt + 1) * FT],
                                wt[:, c * 512:(c + 1) * 512],
                                start=(mt == 0), stop=(mt == NTILES - 1))
                    nc.vector.tensor_mul(z_sb[32 * k:32 * k + FT, :], z_ps[:],
                                         rec_b[:])

        # ================= gcn =================
        with tc.tile_pool(name="gcnp", bufs=1) as gcnp, \
             tc.tile_pool(name="ypp", bufs=2) as ypp, \
             tc.tile_pool(name="ynp", bufs=6) as ynp, \
             tc.tile_pool(name="lns", bufs=6) as lns:
            # gcnB rows 0-63: gcn_pad blocks 0..13 ([0,g0..g11,0]); rows
            # 64-127: gcn_pad blocks shifted by one ([g0..g11,0,-]).
            gcnB = gcnp.tile([128, (T + 2) * N], BF, tag="gcnB")
            nc.vector.memset(gcnB[0:64, 0:N], 0.0)
            nc.vector.memset(gcnB[0:64, (T + 1) * N:(T + 2) * N], 0.0)
            nc.vector.memset(gcnB[64:128, T * N:(T + 1) * N], 0.0)
            with tc.tile_pool(name="gps", bufs=2, space="PSUM") as gps:
                for t in range(T):
                    g_ps = gps.tile([O, N // 2], F32, tag="g_ps")
                    g_ps2 = gps.tile([O, N // 2], F32, tag="g_ps2")
                    for c in range(4):
                        pp = g_ps if c < 2 else g_ps2
                        nc.tensor.matmul(
                            pp[:, (c % 2) * 512:(c % 2 + 1) * 512],
                            thg_s[:, t * O:(t + 1) * O],
                            z_sb[:, c * 512:(c + 1) * 512],
                            start=True, stop=True)
                    # half-relus run on Act and DVE in parallel
                    nc.scalar.activation(
                        gcnB[0:64, (t + 1) * N:(t + 1) * N + 1024],
                        g_ps[:], FN.Relu)
                    nc.vector.tensor_scalar_max(
                        gcnB[0:64, (t + 1) * N + 1024:(t + 2) * N],
                        g_ps2[:], 0.0)
                    # shifted copy for the packed-contraction tconv
                    nc.sync.dma_start(gcnB[64:128, t * N:(t + 1) * N],
                                      gcnB[0:64, (t + 1) * N:(t + 2) * N])

            # ====== tconv + res, with layernorm interleaved per column-chunk
            gfull_s = gcnp.tile([128, OT], BF, tag="gfull_s")
            nc.sync.dma_start(gfull_s[:], gfull[:])
            bfull_s = gcnp.tile([128, OT], BF, tag="bfull_s")
            nc.sync.dma_start(bfull_s[:], bfull[:])
            epsb = gcnp.tile([128, 1], F32, tag="epsb")
            nc.vector.memset(epsb[:], float(O) * LN_EPS)
            with tc.tile_pool(name="tps", bufs=8, space="PSUM") as tps:
                for c in range(4):
                    ypre_s = ypp.tile([O, T * 512], BF, tag="ypre")
                    for t in range(T):
                        tc_ps = tps.tile([O, 512], F32, tag="tc_ps")
                        nc.tensor.matmul(
                            tc_ps[:], twa_s[:],
                            gcnB[:, t * N + c * 512:t * N + (c + 1) * 512],
                            start=True, stop=False)
                        nc.tensor.matmul(
                            tc_ps[:], tw2p_s[64:128, :],
                            gcnB[64:128, (t + 1) * N + c * 512:
                                 (t + 1) * N + (c + 1) * 512],
                            start=False, stop=False)
                        nc.tensor.matmul(
                            tc_ps[:], rwp_s[:, t * O:(t + 1) * O],
                            xtfn_s[:, c * 512:(c + 1) * 512],
                            start=False, stop=True)
                        if t % 2 == 0:
                            nc.scalar.activation(
                                ypre_s[:, t * 512:(t + 1) * 512],
                                tc_ps[:], FN.Relu, bias=biaso_s[:], scale=1.0)
                        else:
                            nc.vector.tensor_scalar(
                                ypre_s[:, t * 512:(t + 1) * 512], tc_ps[:],
                                biaso_s[:], 0.0, op0=OP.add, op1=OP.max)
                    ypdv = ypd_c[c][:].rearrange("(t o) n -> o t n", o=O, t=T)
                    nc.sync.dma_start(
                        ypdv[:, :, :],
                        ypre_s[:].rearrange("o (t n) -> o t n", t=T))
                    # ---- layernorm for the 4 n-tiles of this chunk ----
                    for q in range(4):
                        nt = c * 4 + q
                        yt = ynp.tile([128, OT], BF, tag="yt")
                        nc.sync.dma_start_transpose(
                            yt[:], ypd_c[c][:, q * 128:(q + 1) * 128])
                        ytv = yt[:].rearrange("p (t o) -> p t o", o=O, t=T)
                        mus = lns.tile([128, T], F32, tag="mus")
                        nc.vector.tensor_reduce(mus[:], ytv, axis=AX.X,
                                                op=OP.add)
                        mud = lns.tile([128, T], F32, tag="mud")
                        nc.vector.tensor_scalar_mul(mud[:], mus[:], 1.0 / O)
                        sqf = ynp.tile([128, OT], BF, tag="sqf")
                        seng = nc.vector if (q % 2) == 0 else nc.gpsimd
                        seng.tensor_mul(sqf[:], yt[:], yt[:])
                        sqs = lns.tile([128, T], F32, tag="sqs")
                        nc.vector.tensor_reduce(
                            sqs[:], sqf[:].rearrange("p (t o) -> p t o",
                                                     o=O, t=T),
                            axis=AX.X, op=OP.add)
                        t1 = lns.tile([128, T], F32, tag="t1")
                        nc.vector.tensor_mul(t1[:], mus[:], mus[:])
                        w_t = lns.tile([128, T], F32, tag="w_t")
                        nc.vector.scalar_tensor_tensor(
                            w_t[:], t1[:], -1.0 / O, sqs[:],
                            op0=OP.mult, op1=OP.add)  # 64*var
                        s_t = lns.tile([128, T], F32, tag="s_t")
                        nc.scalar.activation(s_t[:], w_t[:], FN.Sqrt,
                                             bias=epsb[:])
                        r_t = lns.tile([128, T], F32, tag="r_t")
                        nc.vector.reciprocal(r_t[:], s_t[:])  # isig/8
                        nmr = lns.tile([128, T], F32, tag="nmr")
                        nc.vector.scalar_tensor_tensor(
                            nmr[:], mud[:], -1.0, r_t[:],
                            op0=OP.mult, op1=OP.mult)
                        yh = ynp.tile([128, OT], BF, tag="yh")
                        yhv = yh[:].rearrange("p (t o) -> p t o", o=O, t=T)
                        for t in range(T):
                            if t % 2 == 1:
                                nc.scalar.activation(
                                    yhv[:, t, :], ytv[:, t, :], FN.Identity,
                                    bias=nmr[:, t:t + 1], scale=r_t[:, t:t + 1])
                            else:
                                nc.vector.tensor_scalar(
                                    yhv[:, t, :], ytv[:, t, :],
                                    mud[:, t:t + 1], r_t[:, t:t + 1],
                                    op0=OP.subtract, op1=OP.mult)
                        yg = ynp.tile([128, OT], BF, tag="yg")
                        nc.vector.tensor_mul(yg[:], yh[:], gfull_s[:])
                        aeng = nc.gpsimd if (q % 2) == 0 else nc.vector
                        aeng.tensor_add(yg[:], yg[:], bfull_s[:])
                        nc.sync.dma_start(y_d[nt * 128:(nt + 1) * 128, :],
                                          yg[:])
        es2.close()

        es.close()
    nc.compile()
    return nc


def _pack(x_b, consts):
    m = dict(consts)
    xb = np.asarray(x_b, np.float32)
    x_nat = xb.reshape(N, FT)
    x_natt = np.ascontiguousarray(xb.transpose(0, 2, 1)).reshape(N, FT)
    pk = lambda a: np.ascontiguousarray(
        a.reshape(NTILES, 128, FT).transpose(1, 0, 2).reshape(128, NTILES * FT)
    ).astype(bf16)
    m["xnat"] = pk(x_nat)
    m["xnatt"] = pk(x_natt)
    m["xftn"] = np.ascontiguousarray(xb.transpose(1, 2, 0).reshape(FT, N)).astype(bf16)
    m["xtfn"] = np.ascontiguousarray(xb.transpose(2, 1, 0).reshape(FT, N)).astype(bf16)
    return m


def kernel(**inputs):
    if "nc" not in _CACHE:
        _CACHE["nc"] = _build()
    nc = _CACHE["nc"]

    f32 = lambda a: np.asarray(a, np.float32)
    x = f32(inputs["x"])
    U1, U2, U3 = f32(inputs["U1"]), f32(inputs["U2"]), f32(inputs["U3"])
    Vs, bs = f32(inputs["Vs"]), f32(inputs["bs"])
    cheb = f32(inputs["cheb"])
    Theta = f32(inputs["Theta"])
    W1, W2, W3 = f32(inputs["W1"]), f32(inputs["W2"]), f32(inputs["W3"])
    be, Ve = f32(inputs["be"]), f32(inputs["Ve"])
    tw, tb = f32(inputs["tconv_w"]), f32(inputs["tconv_b"])
    rw, rb = f32(inputs["rconv_w"]), f32(inputs["rconv_b"])
    g, bb = f32(inputs["ln_g"]), f32(inputs["ln_b"])

    consts = {}
    consts["u1t"] = np.ascontiguousarray(U1.reshape(NTILES, 128).T).astype(bf16)
    consts["u2t"] = np.ascontiguousarray(
        U2.T.reshape(NTILES, 128, F).transpose(1, 0, 2).reshape(128, NTILES * F)
    ).astype(bf16)
    consts["u3b"] = np.broadcast_to(U3[None, :], (128, 2)).copy()
    consts["w1"] = W1[:, None].astype(bf16)
    consts["w2"] = W2.astype(bf16)
    consts["w3b"] = np.broadcast_to(W3[None, :], (128, 2)).copy()
    consts["bes"] = np.ascontiguousarray(be[0])
    consts["vet"] = np.ascontiguousarray(Ve.T)
    consts["ident"] = np.eye(T, dtype=np.float32)
    idDR = np.zeros((128, 384), np.float32)
    idDR[:, 0:128] = np.eye(128)
    idDR[:, 256:384] = np.eye(128)
    consts["idDR"] = idDR.astype(f8e4)
    consts["idbf"] = np.eye(128, dtype=np.float32).astype(bf16)
    consts["bsb"] = bs[0].astype(f8e4)
    VsT = np.ascontiguousarray(Vs.T)  # [n, m]
    consts["vst"] = np.ascontiguousarray(
        (SV * VsT).reshape(NTILES, 128, NTILES, 128).transpose(2, 1, 0, 3)
        .reshape(NTILES, 128, N)
    ).astype(f8e4)
    consts["rowb"] = np.ascontiguousarray(
        (0.5 * Vs.sum(axis=1)).reshape(NTILES, 128).T).astype(np.float32)
    consts["chebb"] = cheb[1:].astype(bf16)
    thg = np.zeros((96, T * O), np.float32)
    for t in range(T):
        for k in range(K):
            for f in range(F):
                thg[32 * k + t * F + f, t * O:(t + 1) * O] = Theta[k, f]
    consts["thg"] = thg.astype(bf16)
    # twa: [tw_dt0 ; tw_dt1] stacked along contraction; tw2p: tw_dt2 at
    # partitions 64:128; rwp: per-t residual-conv lhsT
    twl = tw[:, :, 0, :].transpose(1, 2, 0).reshape(O, 3 * O)  # [i, (dt, o)]
    consts["twa"] = np.concatenate(
        [twl[:, 0:O], twl[:, O:2 * O]], axis=0).astype(bf16)
    tw2p = np.zeros((128, O), np.float32)
    tw2p[64:128, :] = twl[:, 2 * O:3 * O]
    consts["tw2p"] = tw2p.astype(bf16)
    rwp = np.zeros((FT, T * O), np.float32)
    for t in range(T):
        for f in range(F):
            rwp[t * F + f, t * O:(t + 1) * O] = rw[:, f, 0, 0]
    consts["rwp"] = rwp.astype(bf16)

    consts["biaso"] = (tb + rb)[:, None]
    consts["gfull"] = np.broadcast_to(
        (8.0 * np.tile(g, T))[None, :], (128, OT)).astype(bf16).copy()
    consts["bfull"] = np.broadcast_to(
        np.tile(bb, T)[None, :], (128, OT)).astype(bf16).copy()
    consts["onesb"] = np.ones((128, 1), bf16)

    in_maps = [_pack(x[b], consts) for b in range(B)]
    _CACHE["im"] = in_maps
    res = run_bass_kernel_spmd(nc, in_maps, list(range(B)))
    out = np.stack([
        np.asarray(res.results[b]["y"], np.float32)
        .reshape(N, T, O).transpose(0, 2, 1)
        for b in range(B)
    ])
    return out.astype(np.float32)


# revision 78
# speedup vs baseline: 1.1723x; 1.0160x over previous
import sys
if '/opt/trn_rl_repo' not in sys.path:
    sys.path.insert(0, '/opt/trn_rl_repo')
import numpy as np
import ml_dtypes

import concourse.bass as bass
import concourse.bacc as bacc
import concourse.tile as tile
from concourse import mybir
from concourse.bass_utils import run_bass_kernel_spmd

BF = mybir.dt.bfloat16
F32 = mybir.dt.float32
F8 = mybir.dt.float8e4
FN = mybir.ActivationFunctionType
OP = mybir.AluOpType
AX = mybir.AxisListType
DR = mybir.MatmulPerfMode.DoubleRow

B, N, F, T = 8, 2048, 2, 12
K, O = 3, 64
FT = F * T           # 24
NTILES = N // 128    # 16
OT = O * T           # 768
LN_EPS = 1e-5
SV = 8.0             # scale folded into fp8 Vs (S_pre = psum/(2*SV) + rowb)

bf16 = ml_dtypes.bfloat16
f8e4 = ml_dtypes.float8_e4m3

_CACHE = {}


def _build():
    nc = bacc.Bacc("TRN2", target_bir_lowering=False, debug=False, num_devices=8)

    def din(name, shape, dt=BF):
        return nc.declare_dram_parameter(name, list(shape), dt, isOutput=False)

    xnat = din("xnat", (128, NTILES * FT))        # x[b] [n,(f,t)] tiled: part=n%128
    xnatt = din("xnatt", (128, NTILES * FT))      # x[b] [n,(t,f)] tiled
    xftn = din("xftn", (FT, N))                   # [(f,t), n]
    xtfn = din("xtfn", (FT, N))                   # [(t,f), n]
    u1t = din("u1t", (128, NTILES))
    u2t = din("u2t", (128, NTILES * F))
    u3b = din("u3b", (128, 2), F32)
    w1 = din("w1", (T, 1))
    w2 = din("w2", (F, T))
    w3b = din("w3b", (128, 2), F32)
    bes = din("bes", (T, T), F32)
    vet = din("vet", (T, T), F32)
    ident = din("ident", (T, T), F32)
    bsb = din("bsb", (N, N), F8)                  # fp8 spatial-attn bias
    idDR = din("idDR", (128, 384), F8)            # [I|0|I] for DoubleRow bias-add
    idbf = din("idbf", (128, 128))                # identity bf16 (diag mask)
    vst = din("vst", (NTILES, 128, N), F8)        # per m-tile: SV*Vs^T fp8
    rowb = din("rowb", (128, NTILES), F32)        # 0.5*rowsum(Vs) per m
    chebb = din("chebb", (K - 1, N, N))           # cheb k=1,2 only (k=0 == I)
    thg = din("thg", (96, T * O))                 # zero-padded gcn lhsT per t
    twa = din("twa", (128, O))                    # [dt0|dt1] stacked tconv lhsT
    tw2p = din("tw2p", (128, O))                  # tw_dt2 at partitions 64:128
    rwp = din("rwp", (FT, T * O))                 # zero-padded res lhsT per t
    biaso = din("biaso", (O, 1), F32)
    gfull = din("gfull", (128, OT))               # ln_g in (t,o) order, bf16
    bfull = din("bfull", (128, OT))               # ln_b in (t,o) order, bf16
    onesb = din("onesb", (128, 1))

    y_d = nc.declare_dram_parameter("y", [N, OT], BF, isOutput=True)  # (t,o) order
    # per-512-column scratch chunks [(t,o), 512] for exact LN pipelining deps
    ypd_c = [nc.dram_tensor("ypd%d" % c, [OT, 512], BF) for c in range(4)]

    with tile.TileContext(nc) as tc:
        from contextlib import ExitStack
        es = ExitStack()
        sm = es.enter_context(tc.tile_pool(name="sm", bufs=1))

        def load(dram, shape, dt=BF):
            t_ = sm.tile(list(shape), dt, tag=dram.name + "_s")
            nc.sync.dma_start(t_[:], dram[:])
            return t_

        # attention-critical loads first; the rest are issued after the
        # temporal-attention block so they don't block its tiny DMAs
        xnat_s = load(xnat, (128, NTILES * FT))
        u1t_s = load(u1t, (128, NTILES))
        u2t_s = load(u2t, (128, NTILES * F))
        u3b_s = load(u3b, (128, 2), F32)
        w1_s = load(w1, (T, 1))
        w2_s = load(w2, (F, T))
        w3b_s = load(w3b, (128, 2), F32)
        bes_s = load(bes, (T, T), F32)
        vet_s = load(vet, (T, T), F32)
        id_s = load(ident, (T, T), F32)
        xnatt_s = load(xnatt, (128, NTILES * FT))
        xtfn_s = load(xtfn, (FT, N))
        idDR_s = load(idDR, (128, 384), F8)
        idb_s = load(idbf, (128, 128))
        rowb_s = load(rowb, (128, NTILES), F32)
        thg_s = load(thg, (96, T * O))
        twa_s = load(twa, (128, O))
        tw2p_s = load(tw2p, (128, O))
        rwp_s = load(rwp, (FT, T * O))
        biaso_s = load(biaso, (O, 1), F32)
        ones_s = load(onesb, (128, 1))

        # ================= temporal attention =================
        rhss_s = sm.tile([T, N], BF, tag="rhss")
        l2t_s = sm.tile([T, N], BF, tag="l2t")
        with tc.tile_pool(name="pst", bufs=1, space="PSUM") as pst, \
             tc.tile_pool(name="pat", bufs=1, space="PSUM") as pat, \
             tc.tile_pool(name="attp", bufs=1) as attp:
            xftn_f = []
            for f in range(F):
                t_ = attp.tile([T, N], BF, tag="xftn%d" % f)
                nc.sync.dma_start(t_[:], xftn[f * T:(f + 1) * T, :])
                xftn_f.append(t_)
            # xU1 [1, 24] = sum_n U1[n] * x_nat[n, (f,t)]
            xu1_ps = pst.tile([1, FT], F32, tag="tiny_ps")
            for j in range(NTILES):
                nc.tensor.matmul(xu1_ps[:], u1t_s[:, j:j + 1],
                                 xnat_s[:, j * FT:(j + 1) * FT],
                                 start=(j == 0), stop=(j == NTILES - 1))
            xu1_ft = sm.tile([F, T], F32, tag="xu1ft")
            xu1_row = attp.tile([1, FT], F32, tag="xu1row")
            nc.vector.tensor_copy(xu1_row[:], xu1_ps[:])
            for f in range(F):
                nc.scalar.dma_start(xu1_ft[f:f + 1, :],
                                    xu1_row[0:1, f * T:(f + 1) * T])

            # rhs_t [n, u] (packed [128, (j,u)]) = x[.,0,u]*U3[0] + x[.,1,u]*U3[1]
            rhs_t = attp.tile([128, NTILES * T], BF, tag="rhs_t")
            xf0 = xnat_s[:].rearrange("p (j f t) -> p j f t", j=NTILES, f=F, t=T)
            tmp_rt = attp.tile([128, NTILES * T], BF, tag="tmp_rt")
            rt_v = tmp_rt[:].rearrange("p (j t) -> p j t", j=NTILES, t=T)
            rhs_tv = rhs_t[:].rearrange("p (j t) -> p j t", j=NTILES, t=T)
            nc.vector.tensor_scalar_mul(rt_v, xf0[:, :, 1, :], u3b_s[:, 1:2])
            nc.vector.scalar_tensor_tensor(rhs_tv, xf0[:, :, 0, :], u3b_s[:, 0:1],
                                           rt_v, op0=OP.mult, op1=OP.add)

            # M [f, u] = sum_n U2^T-tiles @ rhs_t
            m_ps = pst.tile([F, T], F32, tag="tiny_ps")
            for j in range(NTILES):
                nc.tensor.matmul(m_ps[:], u2t_s[:, j * F:(j + 1) * F],
                                 rhs_t[:, j * T:(j + 1) * T],
                                 start=(j == 0), stop=(j == NTILES - 1))
            m_s = sm.tile([F, T], F32, tag="m_s")
            nc.vector.tensor_copy(m_s[:], m_ps[:])

            # prod_t [t, u] = xu1_ft^T @ M ; PT = sigmoid(prod_t + be)
            pt_ps = pst.tile([T, T], F32, tag="tiny_ps")
            nc.tensor.matmul(pt_ps[:], xu1_ft[:], m_s[:], start=True, stop=True)
            pt_sb = sm.tile([T, T], F32, tag="pt_sb")
            nc.vector.tensor_add(pt_sb[:], pt_ps[:], bes_s[:])
            nc.scalar.activation(pt_sb[:], pt_sb[:], FN.Sigmoid)

            # E0 [t, v] = Ve @ PT  (lhsT = Ve^T)
            e0_ps = pst.tile([T, T], F32, tag="tiny_ps")
            nc.tensor.matmul(e0_ps[:], vet_s[:], pt_sb[:], start=True, stop=True)
            e0_s = sm.tile([T, T], F32, tag="e0_s")
            nc.vector.tensor_copy(e0_s[:], e0_ps[:])
            e0t_ps = pst.tile([T, T], F32, tag="tiny_ps")
            nc.tensor.transpose(e0t_ps[:], e0_s[:], id_s[:])
            e0t_s = sm.tile([T, T], F32, tag="e0t_s")
            nc.vector.tensor_copy(e0t_s[:], e0t_ps[:])
            # softmax along free (t) for each v
            mx = sm.tile([T, 1], F32, tag="mx")
            nc.vector.tensor_reduce(mx[:], e0t_s[:], axis=AX.X, op=OP.max)
            nmx = sm.tile([T, 1], F32, tag="nmx")
            nc.vector.tensor_scalar_mul(nmx[:], mx[:], -1.0)
            esum = sm.tile([T, 1], F32, tag="esum")
            nc.scalar.activation(e0t_s[:], e0t_s[:], FN.Exp, bias=nmx[:],
                                 scale=1.0, accum_out=esum[:])
            recs = sm.tile([T, 1], F32, tag="recs")
            nc.vector.reciprocal(recs[:], esum[:])
            nc.vector.tensor_scalar_mul(e0t_s[:], e0t_s[:], recs[:])
            esm_ps = pst.tile([T, T], F32, tag="tiny_ps")
            nc.tensor.transpose(esm_ps[:], e0t_s[:], id_s[:])
            esm_bf = sm.tile([T, T], BF, tag="esm_bf")
            nc.vector.tensor_copy(esm_bf[:], esm_ps[:])

            # xTA_ftn [(f,t), n] : per f: Esm^T @ x_ftn[f]
            xta_f = []
            for f in range(F):
                xta_t = attp.tile([T, N], BF, tag="xta%d" % f)
                xta_ps = pat.tile([T, N], F32, tag="attnps")
                for c in range(4):
                    nc.tensor.matmul(xta_ps[:, c * 512:(c + 1) * 512], esm_bf[:],
                                     xftn_f[f][:, c * 512:(c + 1) * 512],
                                     start=True, stop=True)
                nc.vector.tensor_copy(xta_t[:], xta_ps[:])
                xta_f.append(xta_t)

            # ============== spatial attention (small parts) ==============
            tmp_rs = attp.tile([T, N], BF, tag="tmp_rs")
            nc.vector.tensor_scalar_mul(tmp_rs[:], xta_f[1][:],
                                        w3b_s[0:T, 1:2])
            nc.vector.scalar_tensor_tensor(rhss_s[:], xta_f[0][:],
                                           w3b_s[0:T, 0:1], tmp_rs[:],
                                           op0=OP.mult, op1=OP.add)
            lsfn_s = sm.tile([F, N], BF, tag="lsfn")
            for f in range(F):
                ls_ps = pat.tile([1, N], F32, tag="attnps")
                for c in range(4):
                    nc.tensor.matmul(ls_ps[:, c * 512:(c + 1) * 512], w1_s[:],
                                     xta_f[f][:, c * 512:(c + 1) * 512],
                                     start=True, stop=True)
                if f == 0:
                    nc.vector.tensor_copy(lsfn_s[0:1, :], ls_ps[:])
                else:
                    ls1 = attp.tile([1, N], BF, tag="ls1")
                    nc.vector.tensor_copy(ls1[:], ls_ps[:])
                    nc.scalar.dma_start(lsfn_s[1:2, :], ls1[:])
            l2_ps = pat.tile([T, N], F32, tag="attnps")
            for c in range(4):
                nc.tensor.matmul(l2_ps[:, c * 512:(c + 1) * 512], w2_s[:],
                                 lsfn_s[:, c * 512:(c + 1) * 512],
                                 start=True, stop=True)
            nc.vector.tensor_copy(l2t_s[:], l2_ps[:])

        # ========== Pc = tanh(0.5*(prod_s + bs)) in fp8 ==========
        from contextlib import ExitStack as _ES
        es2 = _ES()
        midp = es2.enter_context(tc.tile_pool(name="midp", bufs=1))
        rec_s = midp.tile([1, N], BF, tag="rec_s")
        rec_b = midp.tile([FT, N], BF, tag="rec_b")
        z_sb = midp.tile([96, N], BF, tag="z_sb")
        NPRE = 8  # cheb tiles prefetched into SBUF during the P phase
        with tc.tile_pool(name="Epool", bufs=1) as epool:
            E_s = epool.tile([128, NTILES * N], BF, tag="E")
            chpre = [epool.tile([128, N], BF, tag="chpre%d" % i,
                                name="chpre%d" % i)
                     for i in range(NPRE)]
            with tc.tile_pool(name="Ppool", bufs=1) as ppool:
                Pc_s = ppool.tile([128, NTILES * N], F8, tag="Pc")
                with tc.tile_pool(name="bsstr", bufs=3) as bsstr, \
                     tc.tile_pool(name="pps", bufs=2, space="PSUM") as pps:
                    for nt in range(NTILES):
                        bst = bsstr.tile([128, N], F8, tag="bst")
                        nc.sync.dma_start(bst[:], bsb[nt * 128:(nt + 1) * 128, :])
                        if nt < NPRE:  # prefetch cheb k=1 while DMA is idle
                            nc.sync.dma_start(
                                chpre[nt][:], chebb[0, nt * 128:(nt + 1) * 128, :])
                        pr_ps = pps.tile([128, N], F32, tag="pr_ps")
                        for c in range(4):
                            nc.tensor.matmul(pr_ps[:, c * 512:(c + 1) * 512],
                                             l2t_s[:, nt * 128:(nt + 1) * 128],
                                             rhss_s[:, c * 512:(c + 1) * 512],
                                             start=True, stop=False)
                            # DoubleRow identity-add of bs: lhsT [I|0]/[0|I]
                            base = (c // 2) * 1024
                            idv = idDR_s[:, (c % 2) * 128:(c % 2) * 128 + 256]
                            nc.tensor.matmul(
                                pr_ps[:, c * 512:(c + 1) * 512],
                                idv.rearrange("p (j m) -> p j m", j=2, m=128),
                                bst[:, base:base + 1024].rearrange(
                                    "p (j n) -> p j n", j=2, n=512),
                                start=False, stop=True, perf_mode=DR)
                        nc.scalar.activation(Pc_s[:, nt * N:(nt + 1) * N],
                                             pr_ps[:], FN.Tanh, scale=0.5)

                # == S_pre = SV*Vs @ Pc (fp8 DoubleRow); E = exp(psum/(2SV)+rowb)
                # colsum via DVE accumulate + one GPSIMD partition all-reduce.
                pc_v = Pc_s[:].rearrange("p (j n) -> p j n", j=NTILES, n=N)
                with tc.tile_pool(name="vstr", bufs=4) as vstr, \
                     tc.tile_pool(name="accp", bufs=1) as accp, \
                     tc.tile_pool(name="sps", bufs=4, space="PSUM") as sps:
                    acc_s = accp.tile([128, N], F32, tag="acc")
                    cs_s = accp.tile([128, N], F32, tag="cs")
                    for mt in range(NTILES):
                        vt = vstr.tile([128, N], F8, tag="vt")
                        nc.sync.dma_start(vt[:], vst[mt, :, :])
                        vt_v = vt[:].rearrange("p (j m) -> p j m", j=NTILES, m=128)
                        for h in range(2):
                            s_ps = sps.tile([128, N // 2], F32, tag="s_ps")
                            for c in range(2):
                                col = h * 1024 + c * 512
                                for jp in range(NTILES // 2):
                                    nc.tensor.matmul(
                                        s_ps[:, c * 512:(c + 1) * 512],
                                        vt_v[:, 2 * jp:2 * jp + 2, :],
                                        pc_v[:, 2 * jp:2 * jp + 2,
                                             col:col + 512],
                                        start=(jp == 0),
                                        stop=(jp == NTILES // 2 - 1),
                                        perf_mode=DR)
                            ev = E_s[:, mt * N + h * 1024:
                                     mt * N + (h + 1) * 1024]
                            nc.scalar.activation(ev, s_ps[:], FN.Exp,
                                                 bias=rowb_s[:, mt:mt + 1],
                                                 scale=1.0 / (2.0 * SV))
                        if mt == 0:
                            nc.vector.tensor_copy(
                                acc_s[:], E_s[:, 0:N])
                        else:
                            nc.vector.tensor_add(
                                acc_s[:], acc_s[:], E_s[:, mt * N:(mt + 1) * N])
                    import concourse.bass_isa as bass_isa
                    nc.gpsimd.partition_all_reduce(
                        cs_s[:], acc_s[:], channels=128,
                        reduce_op=bass_isa.ReduceOp.add)
                    with nc.allow_low_precision(reason="colsum ~2048*E; bf16 "
                                                "rec only scales softmax"):
                        nc.vector.reciprocal(rec_s[:], cs_s[0:1, :])
                nc.gpsimd.partition_broadcast(rec_b[:], rec_s[:])

            # ========== cheb: Z_k = x_natt^T @ (cheb_k * E) ==========
            # k=0: cheb_0 == I, so only diagonal blocks of E contribute.
            nc.vector.memset(z_sb[:], 0.0)
            with tc.tile_pool(name="chstr", bufs=10) as chstr, \
                 tc.tile_pool(name="wtmp", bufs=4) as wtmp, \
                 tc.tile_pool(name="zps", bufs=2, space="PSUM") as zps:
                z0_ps = zps.tile([FT, N], F32, tag="z_ps")
                for mt in range(NTILES):
                    wt0 = wtmp.tile([128, 128], BF, tag="wt0")
                    d0 = mt * N + mt * 128
                    nc.vector.tensor_mul(wt0[:], E_s[:, d0:d0 + 128], idb_s[:])
                    nc.tensor.matmul(z0_ps[:, mt * 128:(mt + 1) * 128],
                                     xnatt_s[:, mt * FT:(mt + 1) * FT],
                                     wt0[:], start=True, stop=True)
                nc.vector.tensor_mul(z_sb[0:FT, :], z0_ps[:], rec_b[:])
                for k in range(1, K):
                    z_ps = zps.tile([FT, N], F32, tag="z_ps")
                    for mt in range(NTILES):
                        if k == 1 and mt < NPRE:
                            cht_ap = chpre[mt][:]
                        else:
                            cht = chstr.tile([128, N], BF, tag="cht")
                            nc.sync.dma_start(
                                cht[:], chebb[k - 1, mt * 128:(mt + 1) * 128, :])
                            cht_ap = cht[:]
                        wt = wtmp.tile([128, N], BF, tag="wt")
                        nc.vector.tensor_mul(wt[:], cht_ap,
                                             E_s[:, mt * N:(mt + 1) * N])
                        for c in range(4):
                            nc.tensor.matmul(
                                z_ps[:, c * 512:(c + 1) * 512],
                                xnatt_s[:, mt * FT:(mt + 1) * FT],
                                wt[:, c * 512:(c + 1) * 512],
                                start=(mt == 0), stop=(mt == NTILES - 1))
                    nc.vector.tensor_mul(z_sb[32 * k:32 * k + FT, :], z_ps[:],
                                         rec_b[:])

        # ================= gcn =================
        with tc.tile_pool(name="gcnp", bufs=1) as gcnp, \
             tc.tile_pool(name="ypp", bufs=2) as ypp, \
             tc.tile_pool(name="ynp", bufs=6) as ynp, \
             tc.tile_pool(name="lns", bufs=6) as lns:
            # gcnB rows 0-63: gcn_pad blocks 0..13 ([0,g0..g11,0]); rows
            # 64-127: gcn_pad blocks shifted by one ([g0..g11,0,-]).
            gcnB = gcnp.tile([128, (T + 2) * N], BF, tag="gcnB")
            nc.vector.memset(gcnB[0:64, 0:N], 0.0)
            nc.vector.memset(gcnB[0:64, (T + 1) * N:(T + 2) * N], 0.0)
            nc.vector.memset(gcnB[64:128, T * N:(T + 1) * N], 0.0)
            with tc.tile_pool(name="gps", bufs=2, space="PSUM") as gps:
                for t in range(T):
                    g_ps = gps.tile([O, N // 2], F32, tag="g_ps")
                    g_ps2 = gps.tile([O, N // 2], F32, tag="g_ps2")
                    for c in range(4):
                        pp = g_ps if c < 2 else g_ps2
                        nc.tensor.matmul(
                            pp[:, (c % 2) * 512:(c % 2 + 1) * 512],
                            thg_s[:, t * O:(t + 1) * O],
                            z_sb[:, c * 512:(c + 1) * 512],
                            start=True, stop=True)
                    # half-relus run on Act and DVE in parallel
                    nc.scalar.activation(
                        gcnB[0:64, (t + 1) * N:(t + 1) * N + 1024],
                        g_ps[:], FN.Relu)
                    nc.vector.tensor_scalar_max(
                        gcnB[0:64, (t + 1) * N + 1024:(t + 2) * N],
                        g_ps2[:], 0.0)
                    # shifted copy for the packed-contraction tconv
                    nc.sync.dma_start(gcnB[64:128, t * N:(t + 1) * N],
                                      gcnB[0:64, (t + 1) * N:(t + 2) * N])

            # ====== tconv + res, with layernorm interleaved per column-chunk
            gfull_s = gcnp.tile([128, OT], BF, tag="gfull_s")
            nc.sync.dma_start(gfull_s[:], gfull[:])
            bfull_s = gcnp.tile([128, OT], BF, tag="bfull_s")
            nc.sync.dma_start(bfull_s[:], bfull[:])
            epsb = gcnp.tile([128, 1], F32, tag="epsb")
            nc.vector.memset(epsb[:], float(O) * LN_EPS)
            with tc.tile_pool(name="tps", bufs=8, space="PSUM") as tps:
                for c in range(4):
                    ypre_s = ypp.tile([O, T * 512], BF, tag="ypre")
                    for t in range(T):
                        tc_ps = tps.tile([O, 512], F32, tag="tc_ps")
                        nc.tensor.matmul(
                            tc_ps[:], twa_s[:],
                            gcnB[:, t * N + c * 512:t * N + (c + 1) * 512],
                            start=True, stop=False)
                        nc.tensor.matmul(
                            tc_ps[:], tw2p_s[64:128, :],
                            gcnB[64:128, (t + 1) * N + c * 512:
                                 (t + 1) * N + (c + 1) * 512],
                            start=False, stop=False)
                        nc.tensor.matmul(
                            tc_ps[:], rwp_s[:, t * O:(t + 1) * O],
                            xtfn_s[:, c * 512:(c + 1) * 512],
                            start=False, stop=True)
                        if t % 3 != 0:
                            nc.scalar.activation(
                                ypre_s[:, t * 512:(t + 1) * 512],
                                tc_ps[:], FN.Relu, bias=biaso_s[:], scale=1.0)
                        else:
                            nc.vector.tensor_scalar(
                                ypre_s[:, t * 512:(t + 1) * 512], tc_ps[:],
                                biaso_s[:], 0.0, op0=OP.add, op1=OP.max)
                    ypdv = ypd_c[c][:].rearrange("(t o) n -> o t n", o=O, t=T)
                    nc.sync.dma_start(
                        ypdv[:, :, :],
                        ypre_s[:].rearrange("o (t n) -> o t n", t=T))
                    # ---- layernorm for the 4 n-tiles of this chunk ----
                    for q in range(4):
                        nt = c * 4 + q
                        yt = ynp.tile([128, OT], BF, tag="yt")
                        nc.sync.dma_start_transpose(
                            yt[:], ypd_c[c][:, q * 128:(q + 1) * 128])
                        ytv = yt[:].rearrange("p (t o) -> p t o", o=O, t=T)
                        mus = lns.tile([128, T], F32, tag="mus")
                        nc.vector.tensor_reduce(mus[:], ytv, axis=AX.X,
                                                op=OP.add)
                        mud = lns.tile([128, T], F32, tag="mud")
                        nc.vector.tensor_scalar_mul(mud[:], mus[:], 1.0 / O)
                        sqf = ynp.tile([128, OT], BF, tag="sqf")
                        seng = nc.vector if (q % 2) == 0 else nc.gpsimd
                        seng.tensor_mul(sqf[:], yt[:], yt[:])
                        sqs = lns.tile([128, T], F32, tag="sqs")
                        nc.vector.tensor_reduce(
                            sqs[:], sqf[:].rearrange("p (t o) -> p t o",
                                                     o=O, t=T),
                            axis=AX.X, op=OP.add)
                        t1 = lns.tile([128, T], F32, tag="t1")
                        nc.vector.tensor_mul(t1[:], mus[:], mus[:])
                        w_t = lns.tile([128, T], F32, tag="w_t")
                        nc.vector.scalar_tensor_tensor(
                            w_t[:], t1[:], -1.0 / O, sqs[:],
                            op0=OP.mult, op1=OP.add)  # 64*var
                        s_t = lns.tile([128, T], F32, tag="s_t")
                        nc.scalar.activation(s_t[:], w_t[:], FN.Sqrt,
                                             bias=epsb[:])
                        r_t = lns.tile([128, T], F32, tag="r_t")
                        nc.vector.reciprocal(r_t[:], s_t[:])  # isig/8
                        nmr = lns.tile([128, T], F32, tag="nmr")
                        nc.vector.scalar_tensor_tensor(
                            nmr[:], mud[:], -1.0, r_t[:],
                            op0=OP.mult, op1=OP.mult)
                        yh = ynp.tile([128, OT], BF, tag="yh")
                        yhv = yh[:].rearrange("p (t o) -> p t o", o=O, t=T)
                        for t in range(T):
                            if t % 2 == 1:
                                nc.scalar.activation(
                                    yhv[:, t, :], ytv[:, t, :], FN.Identity,
                                    bias=nmr[:, t:t + 1], scale=r_t[:, t:t + 1])
                            else:
                                nc.vector.tensor_scalar(
                                    yhv[:, t, :], ytv[:, t, :],
                                    mud[:, t:t + 1], r_t[:, t:t + 1],
                                    op0=OP.subtract, op1=OP.mult)
                        yg = ynp.tile([128, OT], BF, tag="yg")
                        nc.vector.tensor_mul(yg[:], yh[:], gfull_s[:])
                        aeng = nc.gpsimd if (q % 2) == 0 else nc.vector
                        aeng.tensor_add(yg[:], yg[:], bfull_s[:])
                        nc.sync.dma_start(y_d[nt * 128:(nt + 1) * 128, :],
                                          yg[:])
        es2.close()

        es.close()
    nc.compile()
    return nc


def _pack(x_b, consts):
    m = dict(consts)
    xb = np.asarray(x_b, np.float32)
    x_nat = xb.reshape(N, FT)
    x_natt = np.ascontiguousarray(xb.transpose(0, 2, 1)).reshape(N, FT)
    pk = lambda a: np.ascontiguousarray(
        a.reshape(NTILES, 128, FT).transpose(1, 0, 2).reshape(128, NTILES * FT)
    ).astype(bf16)
    m["xnat"] = pk(x_nat)
    m["xnatt"] = pk(x_natt)
    m["xftn"] = np.ascontiguousarray(xb.transpose(1, 2, 0).reshape(FT, N)).astype(bf16)
    m["xtfn"] = np.ascontiguousarray(xb.transpose(2, 1, 0).reshape(FT, N)).astype(bf16)
    return m


def kernel(**inputs):
    if "nc" not in _CACHE:
        _CACHE["nc"] = _build()
    nc = _CACHE["nc"]

    f32 = lambda a: np.asarray(a, np.float32)
    x = f32(inputs["x"])
    U1, U2, U3 = f32(inputs["U1"]), f32(inputs["U2"]), f32(inputs["U3"])
    Vs, bs = f32(inputs["Vs"]), f32(inputs["bs"])
    cheb = f32(inputs["cheb"])
    Theta = f32(inputs["Theta"])
    W1, W2, W3 = f32(inputs["W1"]), f32(inputs["W2"]), f32(inputs["W3"])
    be, Ve = f32(inputs["be"]), f32(inputs["Ve"])
    tw, tb = f32(inputs["tconv_w"]), f32(inputs["tconv_b"])
    rw, rb = f32(inputs["rconv_w"]), f32(inputs["rconv_b"])
    g, bb = f32(inputs["ln_g"]), f32(inputs["ln_b"])

    consts = {}
    consts["u1t"] = np.ascontiguousarray(U1.reshape(NTILES, 128).T).astype(bf16)
    consts["u2t"] = np.ascontiguousarray(
        U2.T.reshape(NTILES, 128, F).transpose(1, 0, 2).reshape(128, NTILES * F)
    ).astype(bf16)
    consts["u3b"] = np.broadcast_to(U3[None, :], (128, 2)).copy()
    consts["w1"] = W1[:, None].astype(bf16)
    consts["w2"] = W2.astype(bf16)
    consts["w3b"] = np.broadcast_to(W3[None, :], (128, 2)).copy()
    consts["bes"] = np.ascontiguousarray(be[0])
    consts["vet"] = np.ascontiguousarray(Ve.T)
    consts["ident"] = np.eye(T, dtype=np.float32)
    idDR = np.zeros((128, 384), np.float32)
    idDR[:, 0:128] = np.eye(128)
    idDR[:, 256:384] = np.eye(128)
    consts["idDR"] = idDR.astype(f8e4)
    consts["idbf"] = np.eye(128, dtype=np.float32).astype(bf16)
    consts["bsb"] = bs[0].astype(f8e4)
    VsT = np.ascontiguousarray(Vs.T)  # [n, m]
    consts["vst"] = np.ascontiguousarray(
        (SV * VsT).reshape(NTILES, 128, NTILES, 128).transpose(2, 1, 0, 3)
        .reshape(NTILES, 128, N)
    ).astype(f8e4)
    consts["rowb"] = np.ascontiguousarray(
        (0.5 * Vs.sum(axis=1)).reshape(NTILES, 128).T).astype(np.float32)
    consts["chebb"] = cheb[1:].astype(bf16)
    thg = np.zeros((96, T * O), np.float32)
    for t in range(T):
        for k in range(K):
            for f in range(F):
                thg[32 * k + t * F + f, t * O:(t + 1) * O] = Theta[k, f]
    consts["thg"] = thg.astype(bf16)
    # twa: [tw_dt0 ; tw_dt1] stacked along contraction; tw2p: tw_dt2 at
    # partitions 64:128; rwp: per-t residual-conv lhsT
    twl = tw[:, :, 0, :].transpose(1, 2, 0).reshape(O, 3 * O)  # [i, (dt, o)]
    consts["twa"] = np.concatenate(
        [twl[:, 0:O], twl[:, O:2 * O]], axis=0).astype(bf16)
    tw2p = np.zeros((128, O), np.float32)
    tw2p[64:128, :] = twl[:, 2 * O:3 * O]
    consts["tw2p"] = tw2p.astype(bf16)
    rwp = np.zeros((FT, T * O), np.float32)
    for t in range(T):
        for f in range(F):
            rwp[t * F + f, t * O:(t + 1) * O] = rw[:, f, 0, 0]
    consts["rwp"] = rwp.astype(bf16)

    consts["biaso"] = (tb + rb)[:, None]
    consts["gfull"] = np.broadcast_to(
        (8.0 * np.tile(g, T))[None, :], (128, OT)).astype(bf16).copy()
    consts["bfull"] = np.broadcast_to(
        np.tile(bb, T)[None, :], (128, OT)).astype(bf16).copy()
    consts["onesb"] = np.ones((128, 1), bf16)

    in_maps = [_pack(x[b], consts) for b in range(B)]
    _CACHE["im"] = in_maps
    res = run_bass_kernel_spmd(nc, in_maps, list(range(B)))
    out = np.stack([
        np.asarray(res.results[b]["y"], np.float32)
        .reshape(N, T, O).transpose(0, 2, 1)
        for b in range(B)
    ])
    return out.astype(np.float32)
